# revision 1
# baseline (speedup 1.0000x reference)
"""Causal multi-head attention on 8 Trainium2 NeuronCores.

Problem: B=2, NH=16, T=2048, D=64 fp32.
Sharding: the 32 (batch, head) pairs are split 4-per-core; each core runs its
heads' full causal attention independently (no collectives).

Per-core kernel design (per head):
  - S^T blocks [k=128 partitions, q free] = K_blk @ Q^T via PE (float32r).
    Causality at 128-row granularity: iteration kb only computes q >= 128*kb.
  - Diagonal 128x128 block gets an additive -1e9 upper-strict-triangle mask
    (DVE, in-place in PSUM).
  - exp(S/8) on ScalarE, PSUM -> SBUF (this doubles as the PSUM evacuation).
  - O^T accumulation: PSUM [65, q] += [V | ones]^T_blk @ P^T_blk. Row 64 is
    the softmax denominator (free).
  - Per-512-column-bank epilogue, overlapped with remaining k-blocks: DVE
    copy of O^T to SBUF, PE-transpose back to [q, 65] in-place into the
    retired PSUM bank, one fused broadcast multiply by 1/sums, DMA out.
  - Software-pipelined emission (S matmuls 4 chunk-groups ahead of exp/O;
    narrow k-blocks bin-packed into shared exp calls, order-independent O
    accumulation with tracked bank-completion triggers). TimelineSim models
    91.4 us/core.

The host side only reformats layouts (transpose/pack/shard in numpy); every
FLOP of the attention math runs on device.
"""

import numpy as np

import concourse.mybir as mybir
import concourse.tile as tile
from concourse import bacc
from concourse.bass_utils import run_bass_kernel_spmd

B, NH, T, D = 2, 16, 2048, 64
HPC = 4  # heads per core
NCORES = 8
NKB = T // 128  # 16 k-blocks of 128 rows
F32 = mybir.dt.float32
F32R = mybir.dt.float32r
NEG = -1.0e9

_cached = {}


def _build(reps=1):
    key = ("nc", reps)
    if key in _cached:
        return _cached[key]
    nc = bacc.Bacc("TRN2", target_bir_lowering=False, debug=False)
    # Q^T / K^T: [64, T] (d on partitions)
    qt = nc.dram_tensor("qt", (HPC, D, T), F32R, kind="ExternalInput").ap()
    kt = nc.dram_tensor("kt", (HPC, D, T), F32R, kind="ExternalInput").ap()
    # V augmented with a ones column: [h, p, c, d] = V[h, 128*c + p, d], d=64 -> 1.0
    v = nc.dram_tensor("v", (HPC, 128, NKB, D + 1), F32R, kind="ExternalInput").ap()
    mask = nc.dram_tensor("mask", (128, 128), F32, kind="ExternalInput").ap()
    ident = nc.dram_tensor("ident", (128, 128), F32, kind="ExternalInput").ap()
    # out [h, p, c*64 + d] = O[h, 128*c + p, d]
    o = nc.dram_tensor("o", (HPC, 128, NKB * D), F32, kind="ExternalOutput").ap()

    EXP = mybir.ActivationFunctionType.Exp

    with tile.TileContext(nc) as tc:
        with (
            tc.tile_pool(name="constp", bufs=1) as constp,
            tc.tile_pool(name="qkp", bufs=3) as qkp,
            tc.tile_pool(name="ptp", bufs=6) as ptp,
            tc.tile_pool(name="osbp", bufs=3) as osbp,
            tc.tile_pool(name="spp", bufs=3, space="PSUM") as spp,
            tc.tile_pool(name="opp", bufs=2, space="PSUM") as opp,
        ):
            # tiles here; the DMAs are emitted inside the body AFTER the
            # first head's critical input pieces so they don't head-of-line
            # block the startup
            mask_sb = constp.tile([128, 128], F32)
            id_sb = constp.tile([128, 128], F32)

            def body():
                _emit_body(
                    nc, tc, qt, kt, v, o, mask_sb, id_sb, qkp, ptp, osbp, spp,
                    opp, mask, ident,
                )

            if reps == 1:
                body()
            else:
                with tc.For_i(0, reps, 1):
                    body()

    nc.compile()
    _cached[key] = nc
    return nc


def _emit_body(
    nc, tc, qt, kt, v, o, mask_sb, id_sb, qkp, ptp, osbp, spp, opp, mask, ident
):
    """Software-pipelined emission.

    Each head's q range is processed in two 1024-column passes (A: q<1024,
    B: q>=1024) so the O^T accumulator needs only 2 PSUM banks, leaving 6
    banks for three 1024-wide S^T tiles. S matmuls run
    four chunk-groups ahead of exp/O in the static schedule so ScalarE (the
    bottleneck engine) never waits on PE. The normalize/transpose epilogue is
    emitted per finished 512-column bank so it overlaps remaining k-blocks.
    """
    EXP = mybir.ActivationFunctionType.Exp

    # Chunk groups: each group shares one [128,1024] S^T tile and ONE exp
    # call. Narrow k-blocks are bin-packed together (total width <= 1024) so
    # ScalarE sees fewer, wider calls and the pipeline stays deep near pass
    # ends. O accumulation is order-independent (has_written), so packing may
    # reorder k-blocks; bank-completion triggers track emitted kbs instead.
    # items = [(kb, local_start, c0, c1)].
    PASS_GROUPS = {
        0: [[0], [1], [2], [3], [4, 5], [6, 7]],
        1: [[0], [1], [2], [3], [4], [5], [6], [7], [8], [9], [10],
            [11, 14, 15], [12, 13]],
    }
    chunks = []  # (h, g, items, exp_lo, exp_hi)
    for h in range(HPC):
        for g in (0, 1):
            for grp in PASS_GROUPS[g]:
                items, off = [], 0
                for kb in grp:
                    qs = kb * 128
                    c0 = max(qs, 1024 * g)
                    c1 = 1024 * (g + 1)
                    if len(grp) == 1:
                        items.append((kb, c0 - 1024 * g, c0, c1))
                        chunks.append((h, g, items, c0 - 1024 * g, 1024))
                    else:
                        items.append((kb, off, c0, c1))
                        off += c1 - c0
                if len(grp) > 1:
                    assert off <= 1024
                    chunks.append((h, g, items, 0, off))
    n = len(chunks)

    sb = {}  # h -> (qt_sb, kt_sb, v_sb)
    heads = {}  # h -> dict
    oaccs = {}  # (h, g) -> psum tile [65, 1024]
    sch_tiles = {}  # chunk idx -> (sch, lo, c0, c1)

    # Warm the ACT exp table immediately (overlaps the first input DMAs);
    # memset source so it depends on no DMA.
    warm = osbp.tile([128, 1], F32, tag="warm")
    nc.vector.memset(warm[:], 0.0)
    nc.scalar.activation(warm[:], warm[:], EXP, scale=0.0)

    def load(h, first=False):
        # First two heads load on the sync (HWDGE) queue — fast startup, the
        # store queue is empty anyway. Later prefetches go on the gpsimd
        # (SWDGE) queue so they never head-of-line-block the output stores.
        eng = nc.sync if first else nc.gpsimd
        qt_sb = qkp.tile([D, T], F32R, tag="qt", name=f"qt_sb{h}")
        kt_sb = qkp.tile([D, T], F32R, tag="kt", name=f"kt_sb{h}")
        v_sb = qkp.tile([128, NKB, D + 1], F32R, tag="v", name=f"v_sb{h}")
        if first:
            # fine-grained leading pieces, one per HWDGE ring (sync + scalar)
            # so the two transfers start concurrently and S(0) unblocks ASAP;
            # the rest on the gpsimd queue in parallel.
            nc.scalar.dma_start(kt_sb[:, :128], kt[h, :, :128])
            nc.sync.dma_start(qt_sb[:, :512], qt[h, :, :512])
            nc.gpsimd.dma_start(qt_sb[:, 512 : T // 2], qt[h, :, 512 : T // 2])
            nc.gpsimd.dma_start(kt_sb[:, 128 : T // 2], kt[h, :, 128 : T // 2])
            eng = nc.gpsimd
        else:
            eng.dma_start(kt_sb[:, : T // 2], kt[h, :, : T // 2])
            eng.dma_start(qt_sb[:, : T // 2], qt[h, :, : T // 2])
        eng.dma_start(v_sb[:, : NKB // 2], v[h, :, : NKB // 2])
        eng.dma_start(kt_sb[:, T // 2 :], kt[h, :, T // 2 :])
        eng.dma_start(qt_sb[:, T // 2 :], qt[h, :, T // 2 :])
        eng.dma_start(v_sb[:, NKB // 2 :], v[h, :, NKB // 2 :])
        sb[h] = (qt_sb, kt_sb, v_sb)

    def emit_S(i):
        h, g, items, exp_lo, exp_hi = chunks[i]
        qt_sb, kt_sb, _ = sb[h]
        sch = spp.tile([128, 1024], F32, tag="s", name=f"sch{i}")
        for kb, lst, c0, c1 in items:
            qs = kb * 128
            w = c1 - c0
            p = 0
            while p < w:
                # split at tile-local PSUM bank boundaries
                pl = min(w, ((lst + p) // 512 + 1) * 512 - lst)
                nc.tensor.matmul(
                    sch[:, lst + p : lst + pl],
                    lhsT=kt_sb[:, qs : qs + 128],
                    rhs=qt_sb[:, c0 + p : c0 + pl],
                    start=True,
                    stop=True,
                )
                if p == 0 and c0 == qs:
                    nc.vector.tensor_add(
                        sch[:, lst : lst + 128], sch[:, lst : lst + 128], mask_sb[:]
                    )
                p = pl
        sch_tiles[i] = sch

    def emit_exp_O(i):
        h, g, items, exp_lo, exp_hi = chunks[i]
        _, _, v_sb = sb[h]
        sch = sch_tiles.pop(i)
        ptt = ptp.tile([128, 1024], F32R, tag="pt", name=f"ptt{i}")
        nc.scalar.activation(
            ptt[:, exp_lo:exp_hi], sch[:, exp_lo:exp_hi], EXP, scale=0.125
        )
        if h not in heads:
            heads[h] = {
                "ot": osbp.tile([D + 1, T], F32, tag="ot", name=f"ot_sb{h}"),
                "o": osbp.tile([128, NKB * D], F32, tag="o", name=f"o_sb{h}"),
                "rec": osbp.tile([128, NKB], F32, tag="rec", name=f"rec{h}"),
            }
        for kb, lst, c0, c1 in items:
            p = c0
            while p < c1:
                pe = min(c1, (p // 512 + 1) * 512)
                b = p // 512  # global 512-col bank; one PSUM tile per bank so
                # a retired bank's in-place norm never aliases the live bank
                if (h, b) not in oaccs:
                    # 128 partitions: rows 0-64 hold O^T+sums; emit_bank_norm
                    # transposes in-place into the retired bank with all 128.
                    oaccs[(h, b)] = opp.tile(
                        [128, 512], F32, tag="oacc", name=f"oacc{h}_{b}"
                    )
                nc.tensor.matmul(
                    oaccs[(h, b)][: D + 1, p - 512 * b : pe - 512 * b],
                    lhsT=v_sb[:, kb, :],
                    rhs=ptt[:, lst + (p - c0) : lst + (pe - c0)],
                    start=(kb == 0),
                    stop=(kb == (pe - 1) // 128),
                    skip_group_check=True,
                )
                p = pe

    def emit_bank_copy(h, b):
        # global bank b (cols [512b, 512b+512)) is final; evacuate to SBUF.
        hd = heads[h]
        nc.vector.tensor_copy(
            hd["ot"][:, 512 * b : 512 * (b + 1)], oaccs[(h, b)][: D + 1, :]
        )

    def emit_bank_norm(h, b):
        # Transpose back to [q, 65] IN-PLACE into this bank's retired oacc
        # tile (its own PSUM bank — no slot stolen, no alias with live banks),
        # then normalize by the sums column and store.
        hd = heads[h]
        ot_sb, o_sb, rec = hd["ot"], hd["o"], hd["rec"]
        pso = oaccs[(h, b)][:, : 4 * 65]
        for j in range(4):
            c = 4 * b + j
            nc.tensor.transpose(
                pso[:, 65 * j : 65 * j + 65],
                ot_sb[:, 128 * c : 128 * c + 128],
                id_sb[:65, :65],
            )
        sums = pso.rearrange("p (c d) -> p c d", d=65)[:, :, 64]
        nc.vector.reciprocal(rec[:, 4 * b : 4 * b + 4], sums)
        # one fused multiply for all 4 chunks: in0 strided [128,4,64], in1 =
        # per-chunk reciprocal broadcast along d (step-0 AP)
        nc.vector.tensor_mul(
            o_sb[:, 256 * b : 256 * (b + 1)].rearrange("p (c d) -> p c d", d=64),
            pso.rearrange("p (c d) -> p c d", d=65)[:, :, :64],
            rec[:, 4 * b : 4 * b + 4].unsqueeze(2).to_broadcast((128, 4, 64)),
        )
        nc.sync.dma_start(
            o[h, :, 256 * b : 256 * (b + 1)], o_sb[:, 256 * b : 256 * (b + 1)]
        )

    LOOKAHEAD = 4
    load(0, first=True)
    # consts behind the critical pieces on the sync queue; mask is needed
    # first (chunk-0 diagonal add), identity only at the first bank norm
    nc.sync.dma_start(mask_sb[:], mask[:])
    load(1, first=True)
    nc.sync.dma_start(id_sb[:], ident[:])
    for i in range(min(LOOKAHEAD, n)):
        emit_S(i)
    deferred = {}  # emit-at chunk idx -> (h, b) norm work
    emitted = {}  # (h, g) -> set of emitted kbs
    triggered = set()  # (h, b)
    for i in range(n):
        h, g, items, _, _ = chunks[i]
        emit_exp_O(i)
        if i + LOOKAHEAD < n:
            emit_S(i + LOOKAHEAD)
        for hb in deferred.pop(i, []):
            emit_bank_norm(*hb)
        # Bank b is complete once every kb <= 4b+3 of its pass is emitted
        # (pass A owns banks 0-1, pass B banks 2-3).
        done = emitted.setdefault((h, g), set())
        done.update(kb for kb, _, _, _ in items)
        for b in ((0, 1) if g == 0 else (2, 3)):
            if (h, b) not in triggered and all(
                kb in done for kb in range(4 * b + 4)
            ):
                triggered.add((h, b))
                emit_bank_copy(h, b)
                deferred.setdefault(min(i + 2, n - 1), []).append((h, b))
                if b == 1 and h + 2 < HPC:
                    load(h + 2)
    for i in sorted(deferred):
        for hb in deferred[i]:
            emit_bank_norm(*hb)
    deferred.clear()


def _prep_in_maps(Q, K, V):
    Q = np.asarray(Q, dtype=np.float32).reshape(B * NH, T, D)
    K = np.asarray(K, dtype=np.float32).reshape(B * NH, T, D)
    V = np.asarray(V, dtype=np.float32).reshape(B * NH, T, D)

    mask = np.where(
        np.arange(128)[:, None] <= np.arange(128)[None, :], 0.0, NEG
    ).astype(np.float32)
    ident = np.eye(128, dtype=np.float32)

    in_maps = []
    for c in range(NCORES):
        hs = slice(HPC * c, HPC * (c + 1))
        qt = Q[hs].transpose(0, 2, 1)  # [hpc, 64, T]
        kt = K[hs].transpose(0, 2, 1)
        va = np.concatenate(
            [V[hs], np.ones((HPC, T, 1), dtype=np.float32)], axis=-1
        )  # [hpc, T, 65]
        va = va.reshape(HPC, NKB, 128, D + 1).transpose(0, 2, 1, 3)  # [hpc,128,16,65]
        in_maps.append(
            {
                "qt": np.ascontiguousarray(qt),
                "kt": np.ascontiguousarray(kt),
                "v": np.ascontiguousarray(va),
                "mask": mask,
                "ident": ident,
            }
        )
    return in_maps


def _gather(results):
    out = np.empty((B * NH, T, D), dtype=np.float32)
    for c in range(NCORES):
        oc = results[c]["o"]  # [HPC, 128, NKB*D]
        for s in range(HPC):
            out[HPC * c + s] = (
                oc[s].reshape(128, NKB, D).transpose(1, 0, 2).reshape(T, D)
            )
    return out.reshape(B, NH, T, D)


def _run(in_maps, **kwargs):
    nc = _build()
    return run_bass_kernel_spmd(nc, in_maps, core_ids=list(range(NCORES)), **kwargs)


def kernel(Q, K, V):
    in_maps = _prep_in_maps(Q, K, V)
    res = _run(in_maps)
    return _gather(res.results)



# revision 60
# speedup vs baseline: 2.1043x; 2.1043x over previous
"""Causal multi-head attention on 8 Trainium2 NeuronCores.

Problem: B=2, NH=16, T=2048, D=64 fp32 in/out.
Sharding: the 32 (batch, head) pairs split 4-per-core; each core runs its
heads' full causal attention independently (no collectives).

Per-core kernel design (per head) — dual-engine exp, burst-accumulated O:
  - All matmul operands bf16 (1 PE cycle/row at any width; halves DMA).
  - S^T [s=128 part, q free] built row-major: per k-block one wide strip of
    causal columns (few, wide S matmuls split only at PSUM bank
    boundaries) plus 8-piece diagonal chunks.
  - exp(S/8) split across TWO engines to beat the single-ACT softmax floor:
      * ACT chunks: one wide activation Exp per chunk, PSUM f32 -> bf16.
      * DVE chunks: two custom 8-stage DVE uop-chain ops registered at
        import: g = relu(cubic(x)) ~ exp(x/256) PSUM -> SBUF f32, then
        g^32 * mask -> bf16 P^T (diagonal causal masking fused free;
        row chunks use a broadcast-ones mask). Split ratio tuned so both
        engines finish together (~1.5x one engine's exp throughput).
  - Engines fully decoupled: ACT owns two PSUM S tiles (1024/896 cols),
    DVE owns one (1024; freed after op1 since g lands in SBUF), each
    engine has its own P^T pool. PSUM: O tile + 3 S tiles = 8 banks.
  - O accumulated in [q, d] orientation: out[q=128, 65] +=
    lhsT=P^T_piece[128s, 128q] @ rhs=[V|1][kb][128, 65]; 65 PE cycles per
    piece (vs 128 in the O^T form), no transposes, denominator in col 64.
    PSUM accumulation state is per-bank and dies when any other start=True
    matmul hits the bank, so each q-block's c+1 contributions are emitted
    as ONE uninterrupted burst of matmuls once all its P^T pieces (kept
    alive in SBUF) are ready. q-block 7 splits at the bank boundary;
    q-block 15 lives in sA2's tail bank.
  - O lives in ONE rotating PSUM bank (block c -> 65-col slot c % 7,
    freed by its group's normalize; bursts gate on the slot predecessor's
    normalize emission), freeing a bank to widen ACT's A1 tile to 1536.
  - Normalize: DVE reciprocal + fused broadcast multiply per 7-block
    group (slots are contiguous per group), f32 -> SBUF, DMA out.
  - Static pipeline: each tile's next S refill is emitted right after the
    exp that frees it (its exact WAR gate) so not-yet-ready matmuls never
    clog PE's 4-deep wait queue; PE p-state warmed by dummy matmuls during
    the initial DMA wait; head h+2 inputs prefetched via SWDGE.
    TimelineSim models 66.9 us/core (baseline form: 91.4).

The host side only reformats layouts (transpose/pack/shard in numpy); every
FLOP of the attention math runs on device.
"""

import numpy as np
import ml_dtypes

import concourse.mybir as mybir
import concourse.tile as tile
from concourse import bacc
from concourse.bass_utils import run_bass_kernel_spmd

B, NH, T, D = 2, 16, 2048, 64
HPC = 4  # heads per core
NCORES = 8
NKB = T // 128  # 16 k-blocks
F32 = mybir.dt.float32
BF16 = mybir.dt.bfloat16

# ---------------------------------------------------------------------------
# Custom DVE exp: g = relu(1 + b1 x + x^2 (b2 + b3 x)) ~ exp(x/256) on the
# raw-score range, then P = g^32 * mask = exp(x/8) * mask. Registered into
# concourse.dve_ops at import (rows 17/18 of the 5-bit opcode space).
# ---------------------------------------------------------------------------


def _fit_exp32_coeffs():
    """Cubic least-squares fit of exp(t)-1 on t = x/256, x in +-7.5 sigma of
    the N(0, 64) score distribution, relative-error weighted. a0 pinned at 1
    so ACT-computed exp and DVE-computed exp agree in absolute scale."""
    t = np.linspace(-90.0, 62.0, 6001) / 256.0
    A = np.stack([t, t * t, t**3], axis=1)
    w = np.exp(-t)
    coef, *_ = np.linalg.lstsq(A * w[:, None], (np.exp(t) - 1.0) * w, rcond=None)
    b1, b2, b3 = (float(c) for c in coef)
    g = 1.0 + b1 * t + b2 * t * t + b3 * t**3
    relerr = float(np.abs(g / np.exp(t) - 1.0).max())
    assert 32.0 * relerr < 5e-3, f"poly too loose: {relerr}"
    return b1 / 256.0, b2 / 256.0**2, b3 / 256.0**3


_C1, _C2, _C3 = _fit_exp32_coeffs()


def _register_dve_ops():
    import concourse.dve_ops as DOPS
    from concourse.dve_spec import (
        C0, C1, C2, One, Spec, Src0, Src1, Zero, _has_src1, lower, maxx,
    )
    from concourse.dve_uop import DveOpSpec

    def ref_poly(in0, in1, c0, c1, c2):
        x = in0.astype(np.float32)
        return np.maximum(
            (x * c2 + c1) * (x * x) + (x * c0 + 1.0), 0.0
        ).astype(np.float32)

    def ref_pow32(in0, in1, c0, c1, c2):
        g = in0.astype(np.float32) ** 32
        if in1 is not None:
            m = np.asarray(in1, np.float32)
            g = g.reshape(m.shape) * m
        return g.astype(np.float32)

    body1 = maxx((Src0 * C2 + C1) * (Src0 * Src0) + (Src0 * C0 + One), Zero)
    g2 = Src0 * Src0
    g4 = g2 * g2
    g8 = g4 * g4
    g16 = g8 * g8
    body2 = (g16 * g16) * Src1

    out = []
    for name, spec in (
        ("ANT_EXP32_POLY", Spec(body=body1, reference=ref_poly)),
        ("ANT_POW32_MASK", Spec(body=body2, reference=ref_pow32)),
    ):
        if name in DOPS._SUB_OPCODE_FOR_NAME:
            out.append(next(op for op in DOPS.OPS if op.name == name))
            continue
        row = max(DOPS._SUB_OPCODE_FOR_NAME.values()) + 1
        assert row < 0x20
        shas = {}
        for ver in ("v3", "v4"):
            try:
                shas[ver] = DveOpSpec(
                    name=name, opcode=row, uops=lower(spec, ver=ver),
                    rd1_en=_has_src1(spec),
                ).sha(ver)
            except Exception:
                pass
        op = DOPS.DveOp(name, spec, subdim=False, uops_sha=shas)
        DOPS.OPS.append(op)
        DOPS.CUSTOM_DVE_SPECS[name] = spec
        DOPS._SUB_OPCODE_FOR_NAME[name] = row
        out.append(op)
    return out


EXP32_POLY, POW32_MASK = _register_dve_ops()

# ---------------------------------------------------------------------------
# Static schedule
# ---------------------------------------------------------------------------

TILE_W = {"A1": 1536, "A2": 1024, "Dt": 1024}
DVE_PANELS = {0: 3, 1: 2, 2: 3, 3: 2}  # extra 8-piece panel chunks on DVE
LOOK = 2  # S-refill lookahead (chunks)


def _build_chunks():
    """Per head: (tile, eng, pieces, diag); pieces = [(kb, c0, c1)] covering
    S^T cols [128c0, 128c1) from k-block kb (row-major strips — one wide S
    matmul per 512-col bank span, minimizing PE instruction count). diag
    chunks carry the 8 diagonal 128-col pieces (kb==c) and run on DVE with
    the causal mask fused; row chunks are fully causal. DVE additionally
    takes DVE_PANELS[h] row chunks, spread uniformly, for exp-load
    balance."""
    heads = []
    for h in range(HPC):
        # row strips (kb, c0, c1): off-diagonal cols of k-block kb
        segs = []  # flat stream of (kb, c0, c1) with c1-c0 <= 8
        for kb in range(15):
            c = kb + 1
            while c < 16:
                take = min(8, 16 - c)
                segs.append((kb, c, c + take))
                c += take
        if h == 0:
            # startup: keep the first two chunks inside the leading DMA
            # pieces (qt[:1152], kt[:256]) by deferring row 0/1's far-column
            # tails a few chunks
            early_tails = [g for g in segs[:4] if g[1] >= 9]
            for g in early_tails:
                segs.remove(g)
                segs.insert(8, g)
        # interleave: diagA ~10% in, diagB ~40%, DVE row chunks offset so
        # DVE work never clusters at head boundaries
        total = sum(c1 - c0 for _, c0, c1 in segs)
        ndve = DVE_PANELS[h]
        chunks = []
        atile = 0
        emitted = 0
        dve_frac = {
            4: [0.18, 0.42, 0.62, 0.82],
            3: [0.22, 0.55, 0.80],
            2: [0.25, 0.70],
            1: [0.55],
            0: [],
        }[ndve]
        if h == HPC - 1:
            dve_frac = [f * 0.82 for f in dve_frac]
        dve_pos = [total * f for f in dve_frac]
        diags = [
            (total * 0.08, [(c, c, c + 1) for c in range(8)]),
            (total * (0.30 if h == HPC - 1 else 0.33),
             [(c, c, c + 1) for c in range(8, 16)]),
        ]
        i = 0
        while i < len(segs):
            if diags and emitted >= diags[0][0]:
                chunks.append(("Dt", "dve", diags.pop(0)[1], True))
                continue
            if dve_pos and emitted >= dve_pos[0]:
                t, eng = "Dt", "dve"
                dve_pos.pop(0)
            else:
                t, eng = ("A1", "A2")[atile % 2], "act"
                atile += 1
            cap = TILE_W[t] // 128
            pieces = []
            used = 0
            while i < len(segs) and used < cap:
                kb, c0, c1 = segs[i]
                take = min(cap - used, c1 - c0)
                pieces.append((kb, c0, c0 + take))
                used += take
                if take < c1 - c0:
                    segs[i] = (kb, c0 + take, c1)
                else:
                    i += 1
            emitted += used
            chunks.append((t, eng, pieces, False))
        for _, d in diags:
            chunks.append(("Dt", "dve", d, True))
        heads.append(chunks)
    return heads


HEAD_CHUNKS = _build_chunks()

_cached = {}


def _build(reps=1):
    key = ("nc", reps)
    if key in _cached:
        return _cached[key]
    nc = bacc.Bacc("TRN2", target_bir_lowering=False, debug=False)
    qt = nc.dram_tensor("qt", (HPC, D, T), BF16, kind="ExternalInput").ap()
    kt = nc.dram_tensor("kt", (HPC, D, T), BF16, kind="ExternalInput").ap()
    # v[h, p, kb, :] = [V[h, 128*kb + p, :] | 1.0]
    v = nc.dram_tensor("v", (HPC, 128, NKB, D + 1), BF16, kind="ExternalInput").ap()
    mask = nc.dram_tensor("mask", (128, 128), BF16, kind="ExternalInput").ap()
    # out [h, p, c*64 + d] = O[h, 128*c + p, d]
    o = nc.dram_tensor("o", (HPC, 128, NKB * D), F32, kind="ExternalOutput").ap()

    with tile.TileContext(nc) as tc:
        with (
            tc.tile_pool(name="constp", bufs=1) as constp,
            tc.tile_pool(name="qkp", bufs=3) as qkp,
            tc.tile_pool(name="pta", bufs=18) as pta,
            tc.tile_pool(name="ptd", bufs=8) as ptd,
            tc.tile_pool(name="gbp", bufs=2) as gbp,
            tc.tile_pool(name="osbp", bufs=2) as osbp,
            tc.tile_pool(name="recp", bufs=6) as recp,
            tc.tile_pool(name="spp", bufs=1, space="PSUM") as spp,
        ):
            mask_sb = constp.tile([128, 128], BF16)
            ones_sb = constp.tile([128, 1], BF16)

            def body():
                _emit_body(
                    nc, tc, qt, kt, v, o, mask_sb, ones_sb, qkp, pta, ptd,
                    gbp, osbp, recp, spp, mask,
                )

            if reps == 1:
                body()
            else:
                with tc.For_i(0, reps, 1):
                    body()

    nc.compile()
    _cached[key] = nc
    return nc


def _emit_body(
    nc, tc, qt, kt, v, o, mask_sb, ones_sb, qkp, pta, ptd, gbp, osbp, recp,
    spp, mask
):
    EXP = mybir.ActivationFunctionType.Exp

    # PSUM (bank-granular tiles, 2 banks each = 8 total): oT holds q-blocks
    # 0-14 (65 cols each); q-block 15 lives in sA2's tail (cols 896:961,
    # within its second bank); sA2's S region is 896 cols.
    oT = spp.tile([128, 512], F32, tag="oT", name="oT")
    sA1 = spp.tile([128, 1536], F32, tag="sA1", name="sA1")
    sDt = spp.tile([128, 1024], F32, tag="sDt", name="sDt")
    sA2 = spp.tile([128, 1024], F32, tag="sA2", name="sA2")
    stile = {"A1": sA1, "A2": sA2, "Dt": sDt}

    sb = {}

    def load(h, first=False):
        qt_sb = qkp.tile([D, T], BF16, tag="qt", name=f"qt_sb{h}")
        kt_sb = qkp.tile([D, T], BF16, tag="kt", name=f"kt_sb{h}")
        v_sb = qkp.tile([128, NKB, D + 1], BF16, tag="v", name=f"v_sb{h}")
        if first:
            # everything head-0 needs early rides the two HWDGE rings;
            # only v (first used ~5us in) goes SWDGE
            # qt rides the sync ring alone (chunks 0-1 gate on it); kt
            # rides the scalar ring so neither queues behind the other
            nc.scalar.dma_start(kt_sb[:, :128], kt[h, :, :128])
            nc.sync.dma_start(qt_sb[:, :1152], qt[h, :, :1152])
            nc.sync.dma_start(qt_sb[:, 1152:], qt[h, :, 1152:])
            nc.scalar.dma_start(kt_sb[:, 128:1024], kt[h, :, 128:1024])
            nc.gpsimd.dma_start(v_sb[:, : NKB // 2], v[h, :, : NKB // 2])
            nc.gpsimd.dma_start(kt_sb[:, 1024:], kt[h, :, 1024:])
            nc.gpsimd.dma_start(v_sb[:, NKB // 2 :], v[h, :, NKB // 2 :])
        else:
            g = nc.gpsimd
            g.dma_start(kt_sb[:, :1024], kt[h, :, :1024])
            g.dma_start(qt_sb[:, :1024], qt[h, :, :1024])
            g.dma_start(v_sb[:, : NKB // 2], v[h, :, : NKB // 2])
            g.dma_start(kt_sb[:, 1024:], kt[h, :, 1024:])
            g.dma_start(qt_sb[:, 1024:], qt[h, :, 1024:])
            g.dma_start(v_sb[:, NKB // 2 :], v[h, :, NKB // 2 :])
        sb[h] = (qt_sb, kt_sb, v_sb)

    chunks = []
    for h in range(HPC):
        for t, eng, pieces, diag in HEAD_CHUNKS[h]:
            chunks.append((h, t, eng, pieces, diag))
    n = len(chunks)

    ptts = {}

    def emit_S(i):
        h, t, eng, pieces, diag = chunks[i]
        qt_sb, kt_sb, _ = sb[h]
        st = stile[t]
        off = 0
        for kb, c0, c1 in pieces:
            w = 128 * (c1 - c0)
            p = 0
            while p < w:
                # split at tile-local PSUM bank boundaries
                pl = min(w, ((off + p) // 512 + 1) * 512 - off)
                nc.tensor.matmul(
                    st[:, off + p : off + pl],
                    lhsT=kt_sb[:, 128 * kb : 128 * kb + 128],
                    rhs=qt_sb[:, 128 * c0 + p : 128 * c0 + pl],
                    start=True,
                    stop=True,
                )
                p = pl
            off += w

    def emit_exp(i):
        h, t, eng, pieces, diag = chunks[i]
        st = stile[t]
        w = 128 * sum(c1 - c0 for _, c0, c1 in pieces)
        if eng == "act":
            ptt = pta.tile([128, 1536], BF16, tag="pt", name=f"ptt{i}")
            nc.scalar.activation(ptt[:, :w], st[:, :w], EXP, scale=0.125)
        else:
            ptt = ptd.tile([128, 1024], BF16, tag="pt", name=f"ptt{i}")
            gb = gbp.tile([128, 1024], F32, tag="g", name=f"g{i}")
            nc.vector._custom_dve(
                EXP32_POLY, out=gb[:, :w], in0=st[:, :w],
                s0=_C1, s1=_C2, imm2=_C3,
            )
            if diag:
                in1 = mask_sb[:].unsqueeze(1).to_broadcast((128, len(pieces), 128))
                nc.vector._custom_dve(
                    POW32_MASK,
                    out=ptt[:, :w].rearrange("p (a b) -> p a b", b=128),
                    in0=gb[:, :w].rearrange("p (a b) -> p a b", b=128),
                    in1=in1,
                )
            else:
                nc.vector._custom_dve(
                    POW32_MASK, out=ptt[:, :w], in0=gb[:, :w],
                    in1=ones_sb[:].to_broadcast((128, w)),
                )
        # record piece locations for the per-block O bursts
        off = 0
        for kb, c0, c1 in pieces:
            for c in range(c0, c1):
                piece_loc[(h, kb, c)] = (ptt, off)
                off += 128

    piece_loc = {}

    def emit_burst(h, c):
        # PSUM accumulation state is per-bank and survives only while no
        # other start=True hits the bank, so each q-block's O accumulation
        # is emitted as ONE uninterrupted run of matmuls once every
        # contributing P^T piece is available in SBUF.
        _, _, v_sb = sb[h]
        nkb = c + 1  # rows 0..c-1 plus the diagonal piece (kb == c)
        base = 65 * (c % 7)  # rotating 65-col slot in the single O bank
        for idx in range(nkb):
            kb = idx if idx < c else c
            ptt, off = piece_loc.pop((h, kb, c))
            nc.tensor.matmul(
                oT[:, base : base + 65],
                lhsT=ptt[:, off : off + 128],
                rhs=v_sb[:, kb, :],
                start=idx == 0,
                stop=idx == nkb - 1,
                skip_group_check=True,
            )

    osbs = {}

    def emit_norm(h, g):
        # normalize group g of head h: g0 = q-blocks 0-6, g1 = 7-13,
        # g2 = 14-15. Each group's blocks map to contiguous slots in the
        # single O bank (block c -> slot c % 7), so one reciprocal + one
        # fused broadcast multiply + one DMA per group.
        if h not in osbs:
            osbs[h] = osbp.tile([128, NKB * D], F32, tag="osb", name=f"osb{h}")
        o_sb = osbs[h]
        c0, nr = ((0, 7), (7, 7), (14, 2))[g]
        rec = recp.tile([128, 7], F32, tag="rec", name=f"rec{h}_{g}")
        ot65 = oT[:, : 7 * 65].rearrange("p (c e) -> p c e", e=65)
        nc.vector.reciprocal(rec[:, :nr], ot65[:, :nr, 64])
        nc.vector.tensor_mul(
            o_sb[:, 64 * c0 : 64 * (c0 + nr)].rearrange("p (c d) -> p c d", d=64),
            ot65[:, :nr, :64],
            rec[:, :nr].unsqueeze(2).to_broadcast((128, nr, 64)),
        )
        nc.sync.dma_start(
            o[h, :, 64 * c0 : 64 * (c0 + nr)], o_sb[:, 64 * c0 : 64 * (c0 + nr)]
        )

    # PE p-state warm-up: dummy matmuls on a memset tile ramp the tensor
    # engine clock during the input-DMA wait so real S matmuls start fast
    wmm = recp.tile([128, 128], BF16, tag="wmm")
    nc.vector.memset(wmm[:], 0.0)
    for _ in range(6):
        nc.tensor.matmul(
            sA1[:, :128], lhsT=wmm[:], rhs=wmm[:], start=True, stop=True
        )
    load(0, first=True)
    warm = recp.tile([128, 1], F32, tag="warm")
    nc.vector.memset(warm[:], 0.0)
    nc.vector.memset(ones_sb[:], 1.0)
    nc.scalar.activation(warm[:], warm[:], EXP, scale=0.0)
    nc.scalar.dma_start(mask_sb[:], mask[:])
    load(1, first=True)

    # Per-tile-class S lookahead: ACT tiles deep (LOOK), the DVE tile
    # shallow — a Dt refill in the in-order PE queue waits on DVE's op1
    # (WAR) and would head-block later ACT-tile refills behind it.
    # tile-driven just-in-time S refills: the next chunk on tile T is
    # emitted right after the exp of the current chunk on T (its exact WAR
    # gate), so not-yet-ready matmuls never clog PE's 4-deep wait queue
    next_on_tile = {}
    last_on_tile = {}
    for j, ch in enumerate(chunks):
        if ch[1] in last_on_tile:
            next_on_tile[last_on_tile[ch[1]]] = j
        last_on_tile[ch[1]] = j
    s_done = set()

    def emit_S_once(j):
        if j is not None and j < n and j not in s_done:
            emit_S(j)
            s_done.add(j)

    for tname in ("A1", "A2", "Dt"):
        emit_S_once(next(j for j, ch in enumerate(chunks) if ch[1] == tname))
    remaining = {(h, c): c + 1 for h in range(HPC) for c in range(16)}
    burst_at = {}  # step -> [(h, c)]
    norm_at = {}  # step -> [(h, g)]
    burst_retry = []
    done_norms = set()
    trig = set()
    for i in range(n):
        h, t, eng, pieces, diag = chunks[i]
        emit_exp(i)
        for kb, c0, c1 in pieces:
            for c in range(c0, c1):
                remaining[(h, c)] -= 1
                if remaining[(h, c)] == 0:
                    burst_at.setdefault(i + 1, []).append((h, c))
        # bursts due now are gated on exps already in flight — emit them
        # ahead of this step's S refill (which waits on exp(i)) so PE has
        # ready work during the exp window
        for hg in norm_at.pop(i, []):
            emit_norm(*hg)
            done_norms.add(hg)
        retry = []
        for hc in burst_at.pop(i, []) + burst_retry:
            # the block's O slot (c % 7) is reused across the head; its
            # previous occupant's normalize must be emitted first or this
            # burst's start=True write lands before that read
            hh, c = hc
            if c >= 7:
                prev = (hh, c - 7)
            elif hh > 0:
                prev = (hh - 1, c + 14 if c + 14 <= 15 else c + 7)
            else:
                prev = None
            if prev is not None:
                pg = 0 if prev[1] <= 6 else (1 if prev[1] <= 13 else 2)
                if (prev[0], pg) not in done_norms:
                    retry.append(hc)
                    continue
            emit_burst(*hc)
            if c in (6, 13, 15):
                norm_at.setdefault(i + 1, []).append(
                    (hh, {6: 0, 13: 1, 15: 2}[c])
                )
        burst_retry = retry
        emit_S_once(next_on_tile.get(i))
        if diag and pieces[0][0] == 8 and h + 2 < HPC and ("ld", h + 2) not in trig:
            trig.add(("ld", h + 2))
            load(h + 2)
    pending = burst_retry + [hc for i in sorted(burst_at) for hc in burst_at[i]]
    pending_norms = [hg for i in sorted(norm_at) for hg in norm_at[i]]
    while pending or pending_norms:
        for hg in pending_norms:
            emit_norm(*hg)
            done_norms.add(hg)
        pending_norms = []
        rest = []
        for hh, c in pending:
            if c >= 7:
                prev = (hh, c - 7)
            elif hh > 0:
                prev = (hh - 1, c + 14 if c + 14 <= 15 else c + 7)
            else:
                prev = None
            if prev is not None:
                pg = 0 if prev[1] <= 6 else (1 if prev[1] <= 13 else 2)
                if (prev[0], pg) not in done_norms:
                    rest.append((hh, c))
                    continue
            emit_burst(hh, c)
            if c in (6, 13, 15):
                pending_norms.append((hh, {6: 0, 13: 1, 15: 2}[c]))
        if rest and len(rest) == len(pending) and not pending_norms:
            raise RuntimeError(f"burst deadlock: {rest}")
        pending = rest


def _prep_in_maps(Q, K, V):
    Q = np.asarray(Q, dtype=np.float32).reshape(B * NH, T, D)
    K = np.asarray(K, dtype=np.float32).reshape(B * NH, T, D)
    V = np.asarray(V, dtype=np.float32).reshape(B * NH, T, D)

    mask = np.where(
        np.arange(128)[:, None] <= np.arange(128)[None, :], 1.0, 0.0
    ).astype(ml_dtypes.bfloat16)

    in_maps = []
    for cc in range(NCORES):
        hs = slice(HPC * cc, HPC * (cc + 1))
        qtc = np.ascontiguousarray(Q[hs].transpose(0, 2, 1)).astype(
            ml_dtypes.bfloat16
        )
        ktc = np.ascontiguousarray(K[hs].transpose(0, 2, 1)).astype(
            ml_dtypes.bfloat16
        )
        va = np.concatenate(
            [V[hs], np.ones((HPC, T, 1), dtype=np.float32)], axis=-1
        )
        vc = np.ascontiguousarray(
            va.reshape(HPC, NKB, 128, D + 1).transpose(0, 2, 1, 3)
        ).astype(ml_dtypes.bfloat16)
        in_maps.append({"qt": qtc, "kt": ktc, "v": vc, "mask": mask})
    return in_maps


def _gather(results):
    out = np.empty((B * NH, T, D), dtype=np.float32)
    for cc in range(NCORES):
        oc = results[cc]["o"]  # [HPC, 128, NKB*D]
        for s in range(HPC):
            out[HPC * cc + s] = (
                oc[s].reshape(128, NKB, D).transpose(1, 0, 2).reshape(T, D)
            )
    return out.reshape(B, NH, T, D)


def _run(in_maps, **kwargs):
    nc = _build()
    return run_bass_kernel_spmd(nc, in_maps, core_ids=list(range(NCORES)), **kwargs)


def kernel(Q, K, V):
    in_maps = _prep_in_maps(Q, K, V)
    res = _run(in_maps)
    return _gather(res.results)


# revision 65
# speedup vs baseline: 2.3851x; 1.1335x over previous
"""Causal multi-head attention on 8 Trainium2 NeuronCores.

Problem: B=2, NH=16, T=2048, D=64 fp32 in/out.
Sharding: the 32 (batch, head) pairs split 4-per-core; each core runs its
heads' full causal attention independently (no collectives).

Per-core kernel design (per head) — dual-engine exp, burst-accumulated O:
  - All matmul operands bf16 (1 PE cycle/row at any width; halves DMA).
  - S^T [s=128 part, q free] built row-major: per k-block one wide strip of
    causal columns (few, wide S matmuls split only at PSUM bank
    boundaries) plus 8-piece diagonal chunks.
  - exp(S/8) split across TWO engines to beat the single-ACT softmax floor:
      * ACT chunks: one wide activation Exp per chunk, PSUM f32 -> bf16.
      * DVE chunks: two custom 8-stage DVE uop-chain ops registered at
        import: g = relu(cubic(x)) ~ exp(x/256) PSUM -> SBUF f32, then
        g^32 * mask -> bf16 P^T (diagonal causal masking fused free;
        row chunks use a broadcast-ones mask). Split ratio tuned so both
        engines finish together (~1.5x one engine's exp throughput).
  - Engines fully decoupled: ACT owns two PSUM S tiles (1024/896 cols),
    DVE owns one (1024; freed after op1 since g lands in SBUF), each
    engine has its own P^T pool. PSUM: O tile + 3 S tiles = 8 banks.
  - O accumulated in [q, d] orientation: out[q=128, 65] +=
    lhsT=P^T_piece[128s, 128q] @ rhs=[V|1][kb][128, 65]; 65 PE cycles per
    piece (vs 128 in the O^T form), no transposes, denominator in col 64.
    PSUM accumulation state is per-bank and dies when any other start=True
    matmul hits the bank, so each q-block's c+1 contributions are emitted
    as ONE uninterrupted burst of matmuls once all its P^T pieces (kept
    alive in SBUF) are ready. q-block 7 splits at the bank boundary;
    q-block 15 lives in sA2's tail bank.
  - O lives in ONE rotating PSUM bank (block c -> 65-col slot c % 7,
    freed by its group's normalize; bursts gate on the slot predecessor's
    normalize emission), freeing a bank to widen ACT's A1 tile to 1536.
  - Normalize: DVE reciprocal + fused broadcast multiply per 7-block
    group (slots are contiguous per group), f32 -> SBUF, DMA out.
  - Static pipeline: each tile's next S refill is emitted right after the
    exp that frees it (its exact WAR gate) so not-yet-ready matmuls never
    clog PE's 4-deep wait queue; PE p-state warmed by dummy matmuls during
    the initial DMA wait; head h+2 inputs prefetched via SWDGE.
    TimelineSim models 66.2 us/core (baseline form: 91.4).

The host side only reformats layouts (transpose/pack/shard in numpy); every
FLOP of the attention math runs on device.
"""

import numpy as np
import ml_dtypes

import concourse.mybir as mybir
import concourse.tile as tile
from concourse import bacc
from concourse.bass_utils import run_bass_kernel_spmd

B, NH, T, D = 2, 16, 2048, 64
HPC = 4  # heads per core
NCORES = 8
NKB = T // 128  # 16 k-blocks
F32 = mybir.dt.float32
BF16 = mybir.dt.bfloat16

# ---------------------------------------------------------------------------
# Custom DVE exp: g = relu(1 + b1 x + x^2 (b2 + b3 x)) ~ exp(x/256) on the
# raw-score range, then P = g^32 * mask = exp(x/8) * mask. Registered into
# concourse.dve_ops at import (rows 17/18 of the 5-bit opcode space).
# ---------------------------------------------------------------------------


def _fit_exp32_coeffs():
    """Cubic least-squares fit of exp(t)-1 on t = x/256, x in +-7.5 sigma of
    the N(0, 64) score distribution, relative-error weighted. a0 pinned at 1
    so ACT-computed exp and DVE-computed exp agree in absolute scale."""
    t = np.linspace(-90.0, 62.0, 6001) / 256.0
    A = np.stack([t, t * t, t**3], axis=1)
    w = np.exp(-t)
    coef, *_ = np.linalg.lstsq(A * w[:, None], (np.exp(t) - 1.0) * w, rcond=None)
    b1, b2, b3 = (float(c) for c in coef)
    g = 1.0 + b1 * t + b2 * t * t + b3 * t**3
    relerr = float(np.abs(g / np.exp(t) - 1.0).max())
    assert 32.0 * relerr < 5e-3, f"poly too loose: {relerr}"
    return b1 / 256.0, b2 / 256.0**2, b3 / 256.0**3


_C1, _C2, _C3 = _fit_exp32_coeffs()


def _register_dve_ops():
    import concourse.dve_ops as DOPS
    from concourse.dve_spec import (
        C0, C1, C2, One, Spec, Src0, Src1, Zero, _has_src1, lower, maxx,
    )
    from concourse.dve_uop import DveOpSpec

    def ref_poly(in0, in1, c0, c1, c2):
        x = in0.astype(np.float32)
        return np.maximum(
            (x * c2 + c1) * (x * x) + (x * c0 + 1.0), 0.0
        ).astype(np.float32)

    def ref_pow32(in0, in1, c0, c1, c2):
        g = in0.astype(np.float32) ** 32
        if in1 is not None:
            m = np.asarray(in1, np.float32)
            g = g.reshape(m.shape) * m
        return g.astype(np.float32)

    body1 = maxx((Src0 * C2 + C1) * (Src0 * Src0) + (Src0 * C0 + One), Zero)
    g2 = Src0 * Src0
    g4 = g2 * g2
    g8 = g4 * g4
    g16 = g8 * g8
    body2 = (g16 * g16) * Src1

    out = []
    for name, spec in (
        ("ANT_EXP32_POLY", Spec(body=body1, reference=ref_poly)),
        ("ANT_POW32_MASK", Spec(body=body2, reference=ref_pow32)),
    ):
        if name in DOPS._SUB_OPCODE_FOR_NAME:
            out.append(next(op for op in DOPS.OPS if op.name == name))
            continue
        row = max(DOPS._SUB_OPCODE_FOR_NAME.values()) + 1
        assert row < 0x20
        shas = {}
        for ver in ("v3", "v4"):
            try:
                shas[ver] = DveOpSpec(
                    name=name, opcode=row, uops=lower(spec, ver=ver),
                    rd1_en=_has_src1(spec),
                ).sha(ver)
            except Exception:
                pass
        op = DOPS.DveOp(name, spec, subdim=False, uops_sha=shas)
        DOPS.OPS.append(op)
        DOPS.CUSTOM_DVE_SPECS[name] = spec
        DOPS._SUB_OPCODE_FOR_NAME[name] = row
        out.append(op)
    return out


EXP32_POLY, POW32_MASK = _register_dve_ops()

# ---------------------------------------------------------------------------
# Static schedule
# ---------------------------------------------------------------------------

TILE_W = {"A1": 1536, "A2": 1024, "Dt": 1024}
DVE_PANELS = {0: 3, 1: 2, 2: 3, 3: 2}  # extra 8-piece panel chunks on DVE
LOOK = 2  # S-refill lookahead (chunks)


def _build_chunks():
    """Per head: (tile, eng, pieces, diag); pieces = [(kb, c0, c1)] covering
    S^T cols [128c0, 128c1) from k-block kb (row-major strips — one wide S
    matmul per 512-col bank span, minimizing PE instruction count). diag
    chunks carry the 8 diagonal 128-col pieces (kb==c) and run on DVE with
    the causal mask fused; row chunks are fully causal. DVE additionally
    takes DVE_PANELS[h] row chunks, spread uniformly, for exp-load
    balance."""
    heads = []
    for h in range(HPC):
        # row strips (kb, c0, c1): off-diagonal cols of k-block kb
        segs = []  # flat stream of (kb, c0, c1) with c1-c0 <= 8
        for kb in range(15):
            c = kb + 1
            while c < 16:
                take = min(8, 16 - c)
                segs.append((kb, c, c + take))
                c += take
        if h == 0:
            # startup: keep the first two chunks inside the leading DMA
            # pieces (qt[:1152], kt[:256]) by deferring row 0/1's far-column
            # tails a few chunks
            early_tails = [g for g in segs[:4] if g[1] >= 9]
            for g in early_tails:
                segs.remove(g)
                segs.insert(8, g)
        # interleave: diagA ~10% in, diagB ~40%, DVE row chunks offset so
        # DVE work never clusters at head boundaries
        total = sum(c1 - c0 for _, c0, c1 in segs)
        ndve = DVE_PANELS[h]
        chunks = []
        atile = 0
        emitted = 0
        dve_frac = {
            4: [0.18, 0.42, 0.62, 0.82],
            3: [0.22, 0.55, 0.80],
            2: [0.25, 0.70],
            1: [0.55],
            0: [],
        }[ndve]
        if h == HPC - 1:
            dve_frac = [f * 0.82 for f in dve_frac]
        dve_pos = [total * f for f in dve_frac]
        diags = [
            (total * 0.08, [(c, c, c + 1) for c in range(8)]),
            (total * (0.30 if h == HPC - 1 else 0.33),
             [(c, c, c + 1) for c in range(8, 16)]),
        ]
        i = 0
        while i < len(segs):
            if diags and emitted >= diags[0][0]:
                chunks.append(("Dt", "dve", diags.pop(0)[1], True))
                continue
            if dve_pos and emitted >= dve_pos[0]:
                t, eng = "Dt", "dve"
                dve_pos.pop(0)
            else:
                t, eng = ("A1", "A2")[atile % 2], "act"
                atile += 1
            cap = TILE_W[t] // 128
            pieces = []
            used = 0
            while i < len(segs) and used < cap:
                kb, c0, c1 = segs[i]
                take = min(cap - used, c1 - c0)
                pieces.append((kb, c0, c0 + take))
                used += take
                if take < c1 - c0:
                    segs[i] = (kb, c0 + take, c1)
                else:
                    i += 1
            emitted += used
            chunks.append((t, eng, pieces, False))
        for _, d in diags:
            chunks.append(("Dt", "dve", d, True))
        heads.append(chunks)
    return heads


HEAD_CHUNKS = _build_chunks()

_cached = {}


def _build(reps=1):
    key = ("nc", reps)
    if key in _cached:
        return _cached[key]
    nc = bacc.Bacc("TRN2", target_bir_lowering=False, debug=False)
    qt = nc.dram_tensor("qt", (HPC, D, T), BF16, kind="ExternalInput").ap()
    kt = nc.dram_tensor("kt", (HPC, D, T), BF16, kind="ExternalInput").ap()
    # v[h, p, kb, :] = [V[h, 128*kb + p, :] | 1.0]
    v = nc.dram_tensor("v", (HPC, 128, NKB, D + 1), BF16, kind="ExternalInput").ap()
    mask = nc.dram_tensor("mask", (128, 128), BF16, kind="ExternalInput").ap()
    # out [h, p, c*64 + d] = O[h, 128*c + p, d]
    o = nc.dram_tensor("o", (HPC, 128, NKB * D), F32, kind="ExternalOutput").ap()

    with tile.TileContext(nc) as tc:
        with (
            tc.tile_pool(name="constp", bufs=1) as constp,
            tc.tile_pool(name="qkp", bufs=3) as qkp,
            tc.tile_pool(name="pta", bufs=18) as pta,
            tc.tile_pool(name="ptd", bufs=8) as ptd,
            tc.tile_pool(name="gbp", bufs=2) as gbp,
            tc.tile_pool(name="osbp", bufs=2) as osbp,
            tc.tile_pool(name="recp", bufs=6) as recp,
            tc.tile_pool(name="spp", bufs=1, space="PSUM") as spp,
        ):
            mask_sb = constp.tile([128, 128], BF16)
            ones_sb = constp.tile([128, 1], BF16)

            def body():
                _emit_body(
                    nc, tc, qt, kt, v, o, mask_sb, ones_sb, qkp, pta, ptd,
                    gbp, osbp, recp, spp, mask,
                )

            if reps == 1:
                body()
            else:
                with tc.For_i(0, reps, 1):
                    body()

    nc.compile()
    _cached[key] = nc
    return nc


def _emit_body(
    nc, tc, qt, kt, v, o, mask_sb, ones_sb, qkp, pta, ptd, gbp, osbp, recp,
    spp, mask
):
    EXP = mybir.ActivationFunctionType.Exp

    # PSUM (bank-granular tiles, 2 banks each = 8 total): oT holds q-blocks
    # 0-14 (65 cols each); q-block 15 lives in sA2's tail (cols 896:961,
    # within its second bank); sA2's S region is 896 cols.
    oT = spp.tile([128, 512], F32, tag="oT", name="oT")
    sA1 = spp.tile([128, 1536], F32, tag="sA1", name="sA1")
    sDt = spp.tile([128, 1024], F32, tag="sDt", name="sDt")
    sA2 = spp.tile([128, 1024], F32, tag="sA2", name="sA2")
    stile = {"A1": sA1, "A2": sA2, "Dt": sDt}

    sb = {}

    def load(h, first=False):
        qt_sb = qkp.tile([D, T], BF16, tag="qt", name=f"qt_sb{h}")
        kt_sb = qkp.tile([D, T], BF16, tag="kt", name=f"kt_sb{h}")
        v_sb = qkp.tile([128, NKB, D + 1], BF16, tag="v", name=f"v_sb{h}")
        if first:
            # NOTHING on the scalar ring: its HWDGE generations occupy the
            # ACT sequencer (~1.2us each) and head-of-line block the first
            # exp. qt+mask ride sync (SP), kt rides the vector ring (DVE is
            # idle until the first diag chunk), the rest goes SWDGE.
            nc.sync.dma_start(kt_sb[:, :128], kt[h, :, :128])
            nc.sync.dma_start(qt_sb[:, :1152], qt[h, :, :1152])
            nc.sync.dma_start(kt_sb[:, 128:1024], kt[h, :, 128:1024])
            nc.sync.dma_start(qt_sb[:, 1152:], qt[h, :, 1152:])
            nc.gpsimd.dma_start(v_sb[:, : NKB // 2], v[h, :, : NKB // 2])
            nc.gpsimd.dma_start(kt_sb[:, 1024:], kt[h, :, 1024:])
            nc.gpsimd.dma_start(v_sb[:, NKB // 2 :], v[h, :, NKB // 2 :])
        else:
            g = nc.gpsimd
            g.dma_start(kt_sb[:, :1024], kt[h, :, :1024])
            g.dma_start(qt_sb[:, :1024], qt[h, :, :1024])
            g.dma_start(v_sb[:, : NKB // 2], v[h, :, : NKB // 2])
            g.dma_start(kt_sb[:, 1024:], kt[h, :, 1024:])
            g.dma_start(qt_sb[:, 1024:], qt[h, :, 1024:])
            g.dma_start(v_sb[:, NKB // 2 :], v[h, :, NKB // 2 :])
        sb[h] = (qt_sb, kt_sb, v_sb)

    chunks = []
    for h in range(HPC):
        for t, eng, pieces, diag in HEAD_CHUNKS[h]:
            chunks.append((h, t, eng, pieces, diag))
    n = len(chunks)

    ptts = {}

    def emit_S(i):
        h, t, eng, pieces, diag = chunks[i]
        qt_sb, kt_sb, _ = sb[h]
        st = stile[t]
        off = 0
        for kb, c0, c1 in pieces:
            w = 128 * (c1 - c0)
            p = 0
            while p < w:
                # split at tile-local PSUM bank boundaries
                pl = min(w, ((off + p) // 512 + 1) * 512 - off)
                nc.tensor.matmul(
                    st[:, off + p : off + pl],
                    lhsT=kt_sb[:, 128 * kb : 128 * kb + 128],
                    rhs=qt_sb[:, 128 * c0 + p : 128 * c0 + pl],
                    start=True,
                    stop=True,
                )
                p = pl
            off += w

    def emit_exp(i):
        h, t, eng, pieces, diag = chunks[i]
        st = stile[t]
        w = 128 * sum(c1 - c0 for _, c0, c1 in pieces)
        if eng == "act":
            ptt = pta.tile([128, 1536], BF16, tag="pt", name=f"ptt{i}")
            nc.scalar.activation(ptt[:, :w], st[:, :w], EXP, scale=0.125)
        else:
            ptt = ptd.tile([128, 1024], BF16, tag="pt", name=f"ptt{i}")
            gb = gbp.tile([128, 1024], F32, tag="g", name=f"g{i}")
            nc.vector._custom_dve(
                EXP32_POLY, out=gb[:, :w], in0=st[:, :w],
                s0=_C1, s1=_C2, imm2=_C3,
            )
            if diag:
                in1 = mask_sb[:].unsqueeze(1).to_broadcast((128, len(pieces), 128))
                nc.vector._custom_dve(
                    POW32_MASK,
                    out=ptt[:, :w].rearrange("p (a b) -> p a b", b=128),
                    in0=gb[:, :w].rearrange("p (a b) -> p a b", b=128),
                    in1=in1,
                )
            else:
                nc.vector._custom_dve(
                    POW32_MASK, out=ptt[:, :w], in0=gb[:, :w],
                    in1=ones_sb[:].to_broadcast((128, w)),
                )
        # record piece locations for the per-block O bursts
        off = 0
        for kb, c0, c1 in pieces:
            for c in range(c0, c1):
                piece_loc[(h, kb, c)] = (ptt, off)
                off += 128

    piece_loc = {}

    def emit_burst(h, c):
        # PSUM accumulation state is per-bank and survives only while no
        # other start=True hits the bank, so each q-block's O accumulation
        # is emitted as ONE uninterrupted run of matmuls once every
        # contributing P^T piece is available in SBUF.
        _, _, v_sb = sb[h]
        nkb = c + 1  # rows 0..c-1 plus the diagonal piece (kb == c)
        base = 65 * (c % 7)  # rotating 65-col slot in the single O bank
        for idx in range(nkb):
            kb = idx if idx < c else c
            ptt, off = piece_loc.pop((h, kb, c))
            nc.tensor.matmul(
                oT[:, base : base + 65],
                lhsT=ptt[:, off : off + 128],
                rhs=v_sb[:, kb, :],
                start=idx == 0,
                stop=idx == nkb - 1,
                skip_group_check=True,
            )

    osbs = {}

    def emit_norm(h, g):
        # normalize group g of head h: g0 = q-blocks 0-6, g1 = 7-13,
        # g2 = 14-15. Each group's blocks map to contiguous slots in the
        # single O bank (block c -> slot c % 7), so one reciprocal + one
        # fused broadcast multiply + one DMA per group.
        if h not in osbs:
            osbs[h] = osbp.tile([128, NKB * D], F32, tag="osb", name=f"osb{h}")
        o_sb = osbs[h]
        c0, nr = ((0, 7), (7, 7), (14, 2))[g]
        rec = recp.tile([128, 7], F32, tag="rec", name=f"rec{h}_{g}")
        ot65 = oT[:, : 7 * 65].rearrange("p (c e) -> p c e", e=65)
        nc.vector.reciprocal(rec[:, :nr], ot65[:, :nr, 64])
        nc.vector.tensor_mul(
            o_sb[:, 64 * c0 : 64 * (c0 + nr)].rearrange("p (c d) -> p c d", d=64),
            ot65[:, :nr, :64],
            rec[:, :nr].unsqueeze(2).to_broadcast((128, nr, 64)),
        )
        nc.sync.dma_start(
            o[h, :, 64 * c0 : 64 * (c0 + nr)], o_sb[:, 64 * c0 : 64 * (c0 + nr)]
        )

    # PE p-state warm-up: dummy matmuls on a memset tile ramp the tensor
    # engine clock during the input-DMA wait so real S matmuls start fast
    wmm = recp.tile([128, 128], BF16, tag="wmm")
    nc.vector.memset(wmm[:], 0.0)
    for _ in range(6):
        nc.tensor.matmul(
            sA1[:, :128], lhsT=wmm[:], rhs=wmm[:], start=True, stop=True
        )
    load(0, first=True)
    warm = recp.tile([128, 1], F32, tag="warm")
    nc.vector.memset(warm[:], 0.0)
    nc.vector.memset(ones_sb[:], 1.0)
    nc.scalar.activation(warm[:], warm[:], EXP, scale=0.0)
    nc.sync.dma_start(mask_sb[:], mask[:])
    load(1)

    # Per-tile-class S lookahead: ACT tiles deep (LOOK), the DVE tile
    # shallow — a Dt refill in the in-order PE queue waits on DVE's op1
    # (WAR) and would head-block later ACT-tile refills behind it.
    # tile-driven just-in-time S refills: the next chunk on tile T is
    # emitted right after the exp of the current chunk on T (its exact WAR
    # gate), so not-yet-ready matmuls never clog PE's 4-deep wait queue
    next_on_tile = {}
    last_on_tile = {}
    for j, ch in enumerate(chunks):
        if ch[1] in last_on_tile:
            next_on_tile[last_on_tile[ch[1]]] = j
        last_on_tile[ch[1]] = j
    s_done = set()

    def emit_S_once(j):
        if j is not None and j < n and j not in s_done:
            emit_S(j)
            s_done.add(j)

    for tname in ("A1", "A2", "Dt"):
        emit_S_once(next(j for j, ch in enumerate(chunks) if ch[1] == tname))
    remaining = {(h, c): c + 1 for h in range(HPC) for c in range(16)}
    burst_at = {}  # step -> [(h, c)]
    norm_at = {}  # step -> [(h, g)]
    burst_retry = []
    done_norms = set()
    trig = set()
    for i in range(n):
        h, t, eng, pieces, diag = chunks[i]
        emit_exp(i)
        for kb, c0, c1 in pieces:
            for c in range(c0, c1):
                remaining[(h, c)] -= 1
                if remaining[(h, c)] == 0:
                    burst_at.setdefault(i + 1, []).append((h, c))
        # bursts due now are gated on exps already in flight — emit them
        # ahead of this step's S refill (which waits on exp(i)) so PE has
        # ready work during the exp window
        for hg in norm_at.pop(i, []):
            emit_norm(*hg)
            done_norms.add(hg)
        retry = []
        for hc in burst_at.pop(i, []) + burst_retry:
            # the block's O slot (c % 7) is reused across the head; its
            # previous occupant's normalize must be emitted first or this
            # burst's start=True write lands before that read
            hh, c = hc
            if c >= 7:
                prev = (hh, c - 7)
            elif hh > 0:
                prev = (hh - 1, c + 14 if c + 14 <= 15 else c + 7)
            else:
                prev = None
            if prev is not None:
                pg = 0 if prev[1] <= 6 else (1 if prev[1] <= 13 else 2)
                if (prev[0], pg) not in done_norms:
                    retry.append(hc)
                    continue
            emit_burst(*hc)
            if c in (6, 13, 15):
                norm_at.setdefault(i + 1, []).append(
                    (hh, {6: 0, 13: 1, 15: 2}[c])
                )
        burst_retry = retry
        emit_S_once(next_on_tile.get(i))
        if diag and pieces[0][0] == 8 and h + 2 < HPC and ("ld", h + 2) not in trig:
            trig.add(("ld", h + 2))
            load(h + 2)
    pending = burst_retry + [hc for i in sorted(burst_at) for hc in burst_at[i]]
    pending_norms = [hg for i in sorted(norm_at) for hg in norm_at[i]]
    while pending or pending_norms:
        for hg in pending_norms:
            emit_norm(*hg)
            done_norms.add(hg)
        pending_norms = []
        rest = []
        for hh, c in pending:
            if c >= 7:
                prev = (hh, c - 7)
            elif hh > 0:
                prev = (hh - 1, c + 14 if c + 14 <= 15 else c + 7)
            else:
                prev = None
            if prev is not None:
                pg = 0 if prev[1] <= 6 else (1 if prev[1] <= 13 else 2)
                if (prev[0], pg) not in done_norms:
                    rest.append((hh, c))
                    continue
            emit_burst(hh, c)
            if c in (6, 13, 15):
                pending_norms.append((hh, {6: 0, 13: 1, 15: 2}[c]))
        if rest and len(rest) == len(pending) and not pending_norms:
            raise RuntimeError(f"burst deadlock: {rest}")
        pending = rest


def _prep_in_maps(Q, K, V):
    Q = np.asarray(Q, dtype=np.float32).reshape(B * NH, T, D)
    K = np.asarray(K, dtype=np.float32).reshape(B * NH, T, D)
    V = np.asarray(V, dtype=np.float32).reshape(B * NH, T, D)

    mask = np.where(
        np.arange(128)[:, None] <= np.arange(128)[None, :], 1.0, 0.0
    ).astype(ml_dtypes.bfloat16)

    in_maps = []
    for cc in range(NCORES):
        hs = slice(HPC * cc, HPC * (cc + 1))
        qtc = np.ascontiguousarray(Q[hs].transpose(0, 2, 1)).astype(
            ml_dtypes.bfloat16
        )
        ktc = np.ascontiguousarray(K[hs].transpose(0, 2, 1)).astype(
            ml_dtypes.bfloat16
        )
        va = np.concatenate(
            [V[hs], np.ones((HPC, T, 1), dtype=np.float32)], axis=-1
        )
        vc = np.ascontiguousarray(
            va.reshape(HPC, NKB, 128, D + 1).transpose(0, 2, 1, 3)
        ).astype(ml_dtypes.bfloat16)
        in_maps.append({"qt": qtc, "kt": ktc, "v": vc, "mask": mask})
    return in_maps


def _gather(results):
    out = np.empty((B * NH, T, D), dtype=np.float32)
    for cc in range(NCORES):
        oc = results[cc]["o"]  # [HPC, 128, NKB*D]
        for s in range(HPC):
            out[HPC * cc + s] = (
                oc[s].reshape(128, NKB, D).transpose(1, 0, 2).reshape(T, D)
            )
    return out.reshape(B, NH, T, D)


def _run(in_maps, **kwargs):
    nc = _build()
    return run_bass_kernel_spmd(nc, in_maps, core_ids=list(range(NCORES)), **kwargs)


def kernel(Q, K, V):
    in_maps = _prep_in_maps(Q, K, V)
    res = _run(in_maps)
    return _gather(res.results)


# revision 67
# speedup vs baseline: 2.5925x; 1.0870x over previous
"""Causal multi-head attention on 8 Trainium2 NeuronCores.

Problem: B=2, NH=16, T=2048, D=64 fp32 in/out.
Sharding: the 32 (batch, head) pairs split 4-per-core; each core runs its
heads' full causal attention independently (no collectives).

Per-core kernel design (per head) — dual-engine exp, burst-accumulated O:
  - All matmul operands bf16 (1 PE cycle/row at any width; halves DMA).
  - S^T [s=128 part, q free] built row-major: per k-block one wide strip of
    causal columns (few, wide S matmuls split only at PSUM bank
    boundaries) plus 8-piece diagonal chunks.
  - exp(S/8) split across TWO engines to beat the single-ACT softmax floor:
      * ACT chunks: one wide activation Exp per chunk, PSUM f32 -> bf16.
      * DVE chunks: two custom 8-stage DVE uop-chain ops registered at
        import: g = relu(cubic(x)) ~ exp(x/256) PSUM -> SBUF f32, then
        g^32 * mask -> bf16 P^T (diagonal causal masking fused free;
        row chunks use a broadcast-ones mask). Split ratio tuned so both
        engines finish together (~1.5x one engine's exp throughput).
  - Engines fully decoupled: ACT owns two PSUM S tiles (1024/896 cols),
    DVE owns one (1024; freed after op1 since g lands in SBUF), each
    engine has its own P^T pool. PSUM: O tile + 3 S tiles = 8 banks.
  - O accumulated in [q, d] orientation: out[q=128, 65] +=
    lhsT=P^T_piece[128s, 128q] @ rhs=[V|1][kb][128, 65]; 65 PE cycles per
    piece (vs 128 in the O^T form), no transposes, denominator in col 64.
    PSUM accumulation state is per-bank and dies when any other start=True
    matmul hits the bank, so each q-block's c+1 contributions are emitted
    as ONE uninterrupted burst of matmuls once all its P^T pieces (kept
    alive in SBUF) are ready. q-block 7 splits at the bank boundary;
    q-block 15 lives in sA2's tail bank.
  - O lives in ONE rotating PSUM bank (block c -> 65-col slot c % 7,
    freed by its group's normalize; bursts gate on the slot predecessor's
    normalize emission), freeing a bank to widen ACT's A1 tile to 1536.
  - Normalize: DVE reciprocal + fused broadcast multiply per 7-block
    group (slots are contiguous per group), f32 -> SBUF, DMA out.
  - Static pipeline: each tile's next S refill is emitted right after the
    exp that frees it (its exact WAR gate) so not-yet-ready matmuls never
    clog PE's 4-deep wait queue; PE p-state warmed by dummy matmuls during
    the initial DMA wait; head h+2 inputs prefetched via SWDGE.
    TimelineSim models 66.2 us/core (baseline form: 91.4).

The host side only reformats layouts (transpose/pack/shard in numpy); every
FLOP of the attention math runs on device.
"""

import numpy as np
import ml_dtypes

import concourse.mybir as mybir
import concourse.tile as tile
from concourse import bacc
from concourse.bass_utils import run_bass_kernel_spmd

B, NH, T, D = 2, 16, 2048, 64
HPC = 4  # heads per core
NCORES = 8
NKB = T // 128  # 16 k-blocks
F32 = mybir.dt.float32
BF16 = mybir.dt.bfloat16

# ---------------------------------------------------------------------------
# Custom DVE exp: g = relu(1 + b1 x + x^2 (b2 + b3 x)) ~ exp(x/256) on the
# raw-score range, then P = g^32 * mask = exp(x/8) * mask. Registered into
# concourse.dve_ops at import (rows 17/18 of the 5-bit opcode space).
# ---------------------------------------------------------------------------


def _fit_exp32_coeffs():
    """Cubic least-squares fit of exp(t)-1 on t = x/256, x in +-7.5 sigma of
    the N(0, 64) score distribution, relative-error weighted. a0 pinned at 1
    so ACT-computed exp and DVE-computed exp agree in absolute scale."""
    t = np.linspace(-90.0, 62.0, 6001) / 256.0
    A = np.stack([t, t * t, t**3], axis=1)
    w = np.exp(-t)
    coef, *_ = np.linalg.lstsq(A * w[:, None], (np.exp(t) - 1.0) * w, rcond=None)
    b1, b2, b3 = (float(c) for c in coef)
    g = 1.0 + b1 * t + b2 * t * t + b3 * t**3
    relerr = float(np.abs(g / np.exp(t) - 1.0).max())
    assert 32.0 * relerr < 5e-3, f"poly too loose: {relerr}"
    return b1 / 256.0, b2 / 256.0**2, b3 / 256.0**3


_C1, _C2, _C3 = _fit_exp32_coeffs()


def _register_dve_ops():
    import concourse.dve_ops as DOPS
    from concourse.dve_spec import (
        C0, C1, C2, One, Spec, Src0, Src1, Zero, _has_src1, lower, maxx,
    )
    from concourse.dve_uop import DveOpSpec

    def ref_poly(in0, in1, c0, c1, c2):
        x = in0.astype(np.float32)
        return np.maximum(
            (x * c2 + c1) * (x * x) + (x * c0 + 1.0), 0.0
        ).astype(np.float32)

    def ref_pow32(in0, in1, c0, c1, c2):
        g = in0.astype(np.float32) ** 32
        if in1 is not None:
            m = np.asarray(in1, np.float32)
            g = g.reshape(m.shape) * m
        return g.astype(np.float32)

    body1 = maxx((Src0 * C2 + C1) * (Src0 * Src0) + (Src0 * C0 + One), Zero)
    g2 = Src0 * Src0
    g4 = g2 * g2
    g8 = g4 * g4
    g16 = g8 * g8
    body2 = (g16 * g16) * Src1

    out = []
    for name, spec in (
        ("ANT_EXP32_POLY", Spec(body=body1, reference=ref_poly)),
        ("ANT_POW32_MASK", Spec(body=body2, reference=ref_pow32)),
    ):
        if name in DOPS._SUB_OPCODE_FOR_NAME:
            out.append(next(op for op in DOPS.OPS if op.name == name))
            continue
        row = max(DOPS._SUB_OPCODE_FOR_NAME.values()) + 1
        assert row < 0x20
        shas = {}
        for ver in ("v3", "v4"):
            try:
                shas[ver] = DveOpSpec(
                    name=name, opcode=row, uops=lower(spec, ver=ver),
                    rd1_en=_has_src1(spec),
                ).sha(ver)
            except Exception:
                pass
        op = DOPS.DveOp(name, spec, subdim=False, uops_sha=shas)
        DOPS.OPS.append(op)
        DOPS.CUSTOM_DVE_SPECS[name] = spec
        DOPS._SUB_OPCODE_FOR_NAME[name] = row
        out.append(op)
    return out


EXP32_POLY, POW32_MASK = _register_dve_ops()

# ---------------------------------------------------------------------------
# Static schedule
# ---------------------------------------------------------------------------

TILE_W = {"A1": 1536, "A2": 1024, "Dt": 1024}
DVE_PANELS = {0: 3, 1: 2, 2: 3, 3: 2}  # extra 8-piece panel chunks on DVE
LOOK = 2  # S-refill lookahead (chunks)


def _build_chunks():
    """Per head: (tile, eng, pieces, diag); pieces = [(kb, c0, c1)] covering
    S^T cols [128c0, 128c1) from k-block kb (row-major strips — one wide S
    matmul per 512-col bank span, minimizing PE instruction count). diag
    chunks carry the 8 diagonal 128-col pieces (kb==c) and run on DVE with
    the causal mask fused; row chunks are fully causal. DVE additionally
    takes DVE_PANELS[h] row chunks, spread uniformly, for exp-load
    balance."""
    heads = []
    for h in range(HPC):
        # row strips (kb, c0, c1): off-diagonal cols of k-block kb
        segs = []  # flat stream of (kb, c0, c1) with c1-c0 <= 8
        for kb in range(15):
            c = kb + 1
            while c < 16:
                take = min(8, 16 - c)
                segs.append((kb, c, c + take))
                c += take
        if h == 0:
            # startup: keep the first two chunks inside the leading DMA
            # pieces (qt[:1152], kt[:256]) by deferring row 0/1's far-column
            # tails a few chunks
            early_tails = [g for g in segs[:4] if g[1] >= 9]
            for g in early_tails:
                segs.remove(g)
                segs.insert(8, g)
        # interleave: diagA ~10% in, diagB ~40%, DVE row chunks offset so
        # DVE work never clusters at head boundaries
        total = sum(c1 - c0 for _, c0, c1 in segs)
        ndve = DVE_PANELS[h]
        chunks = []
        atile = 0
        emitted = 0
        dve_frac = {
            4: [0.18, 0.42, 0.62, 0.82],
            3: [0.22, 0.55, 0.80],
            2: [0.25, 0.70],
            1: [0.55],
            0: [],
        }[ndve]
        if h == HPC - 1:
            dve_frac = [f * 0.82 for f in dve_frac]
        dve_pos = [total * f for f in dve_frac]
        diags = [
            (total * 0.08, [(c, c, c + 1) for c in range(8)]),
            (total * (0.30 if h == HPC - 1 else 0.33),
             [(c, c, c + 1) for c in range(8, 16)]),
        ]
        i = 0
        while i < len(segs):
            if diags and emitted >= diags[0][0]:
                chunks.append(("Dt", "dve", diags.pop(0)[1], True))
                continue
            if dve_pos and emitted >= dve_pos[0]:
                t, eng = "Dt", "dve"
                dve_pos.pop(0)
            else:
                t, eng = ("A1", "A2")[atile % 2], "act"
                atile += 1
            cap = TILE_W[t] // 128
            pieces = []
            used = 0
            while i < len(segs) and used < cap:
                kb, c0, c1 = segs[i]
                take = min(cap - used, c1 - c0)
                pieces.append((kb, c0, c0 + take))
                used += take
                if take < c1 - c0:
                    segs[i] = (kb, c0 + take, c1)
                else:
                    i += 1
            emitted += used
            chunks.append((t, eng, pieces, False))
        for _, d in diags:
            chunks.append(("Dt", "dve", d, True))
        heads.append(chunks)
    return heads


HEAD_CHUNKS = _build_chunks()

_cached = {}


def _build(reps=1):
    key = ("nc", reps)
    if key in _cached:
        return _cached[key]
    nc = bacc.Bacc("TRN2", target_bir_lowering=False, debug=False)
    qt = nc.dram_tensor("qt", (HPC, D, T), BF16, kind="ExternalInput").ap()
    kt = nc.dram_tensor("kt", (HPC, D, T), BF16, kind="ExternalInput").ap()
    # v[h, p, kb, :] = [V[h, 128*kb + p, :] | 1.0]
    v = nc.dram_tensor("v", (HPC, 128, NKB, D + 1), BF16, kind="ExternalInput").ap()
    mask = nc.dram_tensor("mask", (128, 128), BF16, kind="ExternalInput").ap()
    # out [h, p, c*64 + d] = O[h, 128*c + p, d]
    o = nc.dram_tensor("o", (HPC, 128, NKB * D), F32, kind="ExternalOutput").ap()

    with tile.TileContext(nc) as tc:
        with (
            tc.tile_pool(name="constp", bufs=1) as constp,
            tc.tile_pool(name="qkp", bufs=3) as qkp,
            tc.tile_pool(name="pta", bufs=18) as pta,
            tc.tile_pool(name="ptd", bufs=8) as ptd,
            tc.tile_pool(name="gbp", bufs=2) as gbp,
            tc.tile_pool(name="osbp", bufs=2) as osbp,
            tc.tile_pool(name="recp", bufs=6) as recp,
            tc.tile_pool(name="spp", bufs=1, space="PSUM") as spp,
        ):
            mask_sb = constp.tile([128, 128], BF16)
            ones_sb = constp.tile([128, 1], BF16)

            def body():
                _emit_body(
                    nc, tc, qt, kt, v, o, mask_sb, ones_sb, qkp, pta, ptd,
                    gbp, osbp, recp, spp, mask,
                )

            if reps == 1:
                body()
            else:
                with tc.For_i(0, reps, 1):
                    body()

    nc.compile()
    _cached[key] = nc
    return nc


def _emit_body(
    nc, tc, qt, kt, v, o, mask_sb, ones_sb, qkp, pta, ptd, gbp, osbp, recp,
    spp, mask
):
    EXP = mybir.ActivationFunctionType.Exp

    # PSUM (bank-granular tiles, 2 banks each = 8 total): oT holds q-blocks
    # 0-14 (65 cols each); q-block 15 lives in sA2's tail (cols 896:961,
    # within its second bank); sA2's S region is 896 cols.
    oT = spp.tile([128, 512], F32, tag="oT", name="oT")
    sA1 = spp.tile([128, 1536], F32, tag="sA1", name="sA1")
    sDt = spp.tile([128, 1024], F32, tag="sDt", name="sDt")
    sA2 = spp.tile([128, 1024], F32, tag="sA2", name="sA2")
    stile = {"A1": sA1, "A2": sA2, "Dt": sDt}

    sb = {}

    def load(h, first=False):
        qt_sb = qkp.tile([D, T], BF16, tag="qt", name=f"qt_sb{h}")
        kt_sb = qkp.tile([D, T], BF16, tag="kt", name=f"kt_sb{h}")
        v_sb = qkp.tile([128, NKB, D + 1], BF16, tag="v", name=f"v_sb{h}")
        if first:
            # NOTHING on the scalar ring: its HWDGE generations occupy the
            # ACT sequencer (~1.2us each) and head-of-line block the first
            # exp. qt+mask ride sync (SP), kt rides the vector ring (DVE is
            # idle until the first diag chunk), the rest goes SWDGE.
            nc.sync.dma_start(kt_sb[:, :128], kt[h, :, :128])
            nc.sync.dma_start(qt_sb[:, :1152], qt[h, :, :1152])
            nc.sync.dma_start(kt_sb[:, 128:1024], kt[h, :, 128:1024])
            nc.sync.dma_start(qt_sb[:, 1152:], qt[h, :, 1152:])
            nc.gpsimd.dma_start(v_sb[:, : NKB // 2], v[h, :, : NKB // 2])
            nc.gpsimd.dma_start(kt_sb[:, 1024:], kt[h, :, 1024:])
            nc.gpsimd.dma_start(v_sb[:, NKB // 2 :], v[h, :, NKB // 2 :])
        else:
            g = nc.gpsimd
            g.dma_start(kt_sb[:, :1024], kt[h, :, :1024])
            g.dma_start(qt_sb[:, :1024], qt[h, :, :1024])
            g.dma_start(v_sb[:, : NKB // 2], v[h, :, : NKB // 2])
            g.dma_start(kt_sb[:, 1024:], kt[h, :, 1024:])
            g.dma_start(qt_sb[:, 1024:], qt[h, :, 1024:])
            g.dma_start(v_sb[:, NKB // 2 :], v[h, :, NKB // 2 :])
        sb[h] = (qt_sb, kt_sb, v_sb)

    chunks = []
    for h in range(HPC):
        for t, eng, pieces, diag in HEAD_CHUNKS[h]:
            chunks.append((h, t, eng, pieces, diag))
    n = len(chunks)

    ptts = {}

    def emit_S(i):
        h, t, eng, pieces, diag = chunks[i]
        qt_sb, kt_sb, _ = sb[h]
        st = stile[t]
        off = 0
        for kb, c0, c1 in pieces:
            w = 128 * (c1 - c0)
            p = 0
            while p < w:
                # split at tile-local PSUM bank boundaries
                pl = min(w, ((off + p) // 512 + 1) * 512 - off)
                nc.tensor.matmul(
                    st[:, off + p : off + pl],
                    lhsT=kt_sb[:, 128 * kb : 128 * kb + 128],
                    rhs=qt_sb[:, 128 * c0 + p : 128 * c0 + pl],
                    start=True,
                    stop=True,
                )
                p = pl
            off += w

    def emit_exp(i):
        h, t, eng, pieces, diag = chunks[i]
        st = stile[t]
        w = 128 * sum(c1 - c0 for _, c0, c1 in pieces)
        if eng == "act":
            ptt = pta.tile([128, 1536], BF16, tag="pt", name=f"ptt{i}")
            nc.scalar.activation(ptt[:, :w], st[:, :w], EXP, scale=0.125)
        else:
            ptt = ptd.tile([128, 1024], BF16, tag="pt", name=f"ptt{i}")
            gb = gbp.tile([128, 1024], F32, tag="g", name=f"g{i}")
            nc.vector._custom_dve(
                EXP32_POLY, out=gb[:, :w], in0=st[:, :w],
                s0=_C1, s1=_C2, imm2=_C3,
            )
            if diag:
                in1 = mask_sb[:].unsqueeze(1).to_broadcast((128, len(pieces), 128))
                nc.vector._custom_dve(
                    POW32_MASK,
                    out=ptt[:, :w].rearrange("p (a b) -> p a b", b=128),
                    in0=gb[:, :w].rearrange("p (a b) -> p a b", b=128),
                    in1=in1,
                )
            else:
                nc.vector._custom_dve(
                    POW32_MASK, out=ptt[:, :w], in0=gb[:, :w],
                    in1=ones_sb[:].to_broadcast((128, w)),
                )
        # record piece locations for the per-block O bursts
        off = 0
        for kb, c0, c1 in pieces:
            for c in range(c0, c1):
                piece_loc[(h, kb, c)] = (ptt, off)
                off += 128

    piece_loc = {}

    def emit_burst(h, c):
        # PSUM accumulation state is per-bank and survives only while no
        # other start=True hits the bank, so each q-block's O accumulation
        # is emitted as ONE uninterrupted run of matmuls once every
        # contributing P^T piece is available in SBUF.
        _, _, v_sb = sb[h]
        nkb = c + 1  # rows 0..c-1 plus the diagonal piece (kb == c)
        base = 65 * (c % 7)  # rotating 65-col slot in the single O bank
        for idx in range(nkb):
            kb = idx if idx < c else c
            ptt, off = piece_loc.pop((h, kb, c))
            nc.tensor.matmul(
                oT[:, base : base + 65],
                lhsT=ptt[:, off : off + 128],
                rhs=v_sb[:, kb, :],
                start=idx == 0,
                stop=idx == nkb - 1,
                skip_group_check=True,
            )

    osbs = {}

    def emit_norm(h, g):
        # normalize group g of head h: g0 = q-blocks 0-6, g1 = 7-13,
        # g2 = 14-15. Each group's blocks map to contiguous slots in the
        # single O bank (block c -> slot c % 7), so one reciprocal + one
        # fused broadcast multiply + one DMA per group.
        if h not in osbs:
            osbs[h] = osbp.tile([128, NKB * D], F32, tag="osb", name=f"osb{h}")
        o_sb = osbs[h]
        c0, nr = ((0, 7), (7, 7), (14, 2))[g]
        rec = recp.tile([128, 7], F32, tag="rec", name=f"rec{h}_{g}")
        ot65 = oT[:, : 7 * 65].rearrange("p (c e) -> p c e", e=65)
        nc.vector.reciprocal(rec[:, :nr], ot65[:, :nr, 64])
        nc.vector.tensor_mul(
            o_sb[:, 64 * c0 : 64 * (c0 + nr)].rearrange("p (c d) -> p c d", d=64),
            ot65[:, :nr, :64],
            rec[:, :nr].unsqueeze(2).to_broadcast((128, nr, 64)),
        )
        nc.sync.dma_start(
            o[h, :, 64 * c0 : 64 * (c0 + nr)], o_sb[:, 64 * c0 : 64 * (c0 + nr)]
        )

    # PE p-state warm-up: dummy matmuls on a memset tile ramp the tensor
    # engine clock during the input-DMA wait so real S matmuls start fast
    wmm = recp.tile([128, 128], BF16, tag="wmm")
    nc.vector.memset(wmm[:], 0.0)
    for _ in range(6):
        nc.tensor.matmul(
            sA1[:, :128], lhsT=wmm[:], rhs=wmm[:], start=True, stop=True
        )
    load(0, first=True)
    warm = recp.tile([128, 1], F32, tag="warm")
    nc.vector.memset(warm[:], 0.0)
    nc.vector.memset(ones_sb[:], 1.0)
    nc.scalar.activation(warm[:], warm[:], EXP, scale=0.0)
    nc.sync.dma_start(mask_sb[:], mask[:])
    load(1)

    # Per-tile-class S lookahead: ACT tiles deep (LOOK), the DVE tile
    # shallow — a Dt refill in the in-order PE queue waits on DVE's op1
    # (WAR) and would head-block later ACT-tile refills behind it.
    # tile-driven just-in-time S refills: the next chunk on tile T is
    # emitted right after the exp of the current chunk on T (its exact WAR
    # gate), so not-yet-ready matmuls never clog PE's 4-deep wait queue
    next_on_tile = {}
    last_on_tile = {}
    for j, ch in enumerate(chunks):
        if ch[1] in last_on_tile:
            next_on_tile[last_on_tile[ch[1]]] = j
        last_on_tile[ch[1]] = j
    s_done = set()

    def emit_S_once(j):
        if j is not None and j < n and j not in s_done:
            emit_S(j)
            s_done.add(j)

    for tname in ("A1", "A2", "Dt"):
        emit_S_once(next(j for j, ch in enumerate(chunks) if ch[1] == tname))
    remaining = {(h, c): c + 1 for h in range(HPC) for c in range(16)}
    burst_at = {}  # step -> [(h, c)]
    norm_at = {}  # step -> [(h, g)]
    burst_retry = []
    done_norms = set()
    trig = set()
    for i in range(n):
        h, t, eng, pieces, diag = chunks[i]
        emit_exp(i)
        for kb, c0, c1 in pieces:
            for c in range(c0, c1):
                remaining[(h, c)] -= 1
                if remaining[(h, c)] == 0:
                    burst_at.setdefault(i + 1, []).append((h, c))
        # bursts due now are gated on exps already in flight — emit them
        # ahead of this step's S refill (which waits on exp(i)) so PE has
        # ready work during the exp window
        for hg in norm_at.pop(i, []):
            emit_norm(*hg)
            done_norms.add(hg)
        retry = []
        for hc in burst_at.pop(i, []) + burst_retry:
            # the block's O slot (c % 7) is reused across the head; its
            # previous occupant's normalize must be emitted first or this
            # burst's start=True write lands before that read
            hh, c = hc
            if c >= 7:
                prev = (hh, c - 7)
            elif hh > 0:
                prev = (hh - 1, c + 14 if c + 14 <= 15 else c + 7)
            else:
                prev = None
            if prev is not None:
                pg = 0 if prev[1] <= 6 else (1 if prev[1] <= 13 else 2)
                if (prev[0], pg) not in done_norms:
                    retry.append(hc)
                    continue
            emit_burst(*hc)
            if c in (6, 13, 15):
                norm_at.setdefault(i + 1, []).append(
                    (hh, {6: 0, 13: 1, 15: 2}[c])
                )
        burst_retry = retry
        emit_S_once(next_on_tile.get(i))
        if diag and pieces[0][0] == 8 and h + 2 < HPC and ("ld", h + 2) not in trig:
            trig.add(("ld", h + 2))
            load(h + 2)
    pending = burst_retry + [hc for i in sorted(burst_at) for hc in burst_at[i]]
    pending_norms = [hg for i in sorted(norm_at) for hg in norm_at[i]]
    while pending or pending_norms:
        for hg in pending_norms:
            emit_norm(*hg)
            done_norms.add(hg)
        pending_norms = []
        rest = []
        for hh, c in pending:
            if c >= 7:
                prev = (hh, c - 7)
            elif hh > 0:
                prev = (hh - 1, c + 14 if c + 14 <= 15 else c + 7)
            else:
                prev = None
            if prev is not None:
                pg = 0 if prev[1] <= 6 else (1 if prev[1] <= 13 else 2)
                if (prev[0], pg) not in done_norms:
                    rest.append((hh, c))
                    continue
            emit_burst(hh, c)
            if c in (6, 13, 15):
                pending_norms.append((hh, {6: 0, 13: 1, 15: 2}[c]))
        if rest and len(rest) == len(pending) and not pending_norms:
            raise RuntimeError(f"burst deadlock: {rest}")
        pending = rest


def _prep_in_maps(Q, K, V):
    Q = np.asarray(Q, dtype=np.float32).reshape(B * NH, T, D)
    K = np.asarray(K, dtype=np.float32).reshape(B * NH, T, D)
    V = np.asarray(V, dtype=np.float32).reshape(B * NH, T, D)

    mask = np.where(
        np.arange(128)[:, None] <= np.arange(128)[None, :], 1.0, 0.0
    ).astype(ml_dtypes.bfloat16)

    in_maps = []
    for cc in range(NCORES):
        hs = slice(HPC * cc, HPC * (cc + 1))
        qtc = np.ascontiguousarray(Q[hs].transpose(0, 2, 1)).astype(
            ml_dtypes.bfloat16
        )
        ktc = np.ascontiguousarray(K[hs].transpose(0, 2, 1)).astype(
            ml_dtypes.bfloat16
        )
        va = np.concatenate(
            [V[hs], np.ones((HPC, T, 1), dtype=np.float32)], axis=-1
        )
        vc = np.ascontiguousarray(
            va.reshape(HPC, NKB, 128, D + 1).transpose(0, 2, 1, 3)
        ).astype(ml_dtypes.bfloat16)
        in_maps.append({"qt": qtc, "kt": ktc, "v": vc, "mask": mask})
    return in_maps


def _gather(results):
    out = np.empty((B * NH, T, D), dtype=np.float32)
    for cc in range(NCORES):
        oc = results[cc]["o"]  # [HPC, 128, NKB*D]
        for s in range(HPC):
            out[HPC * cc + s] = (
                oc[s].reshape(128, NKB, D).transpose(1, 0, 2).reshape(T, D)
            )
    return out.reshape(B, NH, T, D)


def _run(in_maps, **kwargs):
    nc = _build()
    return run_bass_kernel_spmd(nc, in_maps, core_ids=list(range(NCORES)), **kwargs)


def kernel(Q, K, V):
    in_maps = _prep_in_maps(Q, K, V)
    res = _run(in_maps)
    return _gather(res.results)


# revision 68
# speedup vs baseline: 4.4586x; 1.7198x over previous
"""Causal multi-head attention on 8 Trainium2 NeuronCores.

Problem: B=2, NH=16, T=2048, D=64 fp32 in/out.
Sharding: the 32 (batch, head) pairs split 4-per-core; each core runs its
heads' full causal attention independently (no collectives).

Per-core kernel design (per head) — dual-engine exp, burst-accumulated O:
  - All matmul operands bf16 (1 PE cycle/row at any width; halves DMA).
  - S^T [s=128 part, q free] built row-major: per k-block one wide strip of
    causal columns (few, wide S matmuls split only at PSUM bank
    boundaries) plus 8-piece diagonal chunks.
  - exp(S/8) split across TWO engines to beat the single-ACT softmax floor:
      * ACT chunks: one wide activation Exp per chunk, PSUM f32 -> bf16.
      * DVE chunks: two custom 8-stage DVE uop-chain ops registered at
        import: g = relu(cubic(x)) ~ exp(x/256) PSUM -> SBUF f32, then
        g^32 * mask -> bf16 P^T (diagonal causal masking fused free;
        row chunks use a broadcast-ones mask). Split ratio tuned so both
        engines finish together (~1.5x one engine's exp throughput).
  - Engines fully decoupled: ACT owns two PSUM S tiles (1024/896 cols),
    DVE owns one (1024; freed after op1 since g lands in SBUF), each
    engine has its own P^T pool. PSUM: O tile + 3 S tiles = 8 banks.
  - O accumulated in [q, d] orientation: out[q=128, 65] +=
    lhsT=P^T_piece[128s, 128q] @ rhs=[V|1][kb][128, 65]; 65 PE cycles per
    piece (vs 128 in the O^T form), no transposes, denominator in col 64.
    PSUM accumulation state is per-bank and dies when any other start=True
    matmul hits the bank, so each q-block's c+1 contributions are emitted
    as ONE uninterrupted burst of matmuls once all its P^T pieces (kept
    alive in SBUF) are ready. q-block 7 splits at the bank boundary;
    q-block 15 lives in sA2's tail bank.
  - O lives in ONE rotating PSUM bank (block c -> 65-col slot c % 7,
    freed by its group's normalize; bursts gate on the slot predecessor's
    normalize emission), freeing a bank to widen ACT's A1 tile to 1536.
  - Normalize: DVE reciprocal + fused broadcast multiply per 7-block
    group (slots are contiguous per group), f32 -> SBUF, DMA out.
  - Static pipeline: each tile's next S refill is emitted right after the
    exp that frees it (its exact WAR gate) so not-yet-ready matmuls never
    clog PE's 4-deep wait queue; PE p-state warmed by dummy matmuls during
    the initial DMA wait; head h+2 inputs prefetched via SWDGE.
    TimelineSim models 66.2 us/core (baseline form: 91.4).

The host side only reformats layouts (transpose/pack/shard in numpy); every
FLOP of the attention math runs on device.
"""

import numpy as np
import ml_dtypes

import concourse.mybir as mybir
import concourse.tile as tile
from concourse import bacc
from concourse.bass_utils import run_bass_kernel_spmd

B, NH, T, D = 2, 16, 2048, 64
HPC = 4  # heads per core
NCORES = 8
NKB = T // 128  # 16 k-blocks
F32 = mybir.dt.float32
BF16 = mybir.dt.bfloat16

# ---------------------------------------------------------------------------
# Custom DVE exp: g = relu(1 + b1 x + x^2 (b2 + b3 x)) ~ exp(x/256) on the
# raw-score range, then P = g^32 * mask = exp(x/8) * mask. Registered into
# concourse.dve_ops at import (rows 17/18 of the 5-bit opcode space).
# ---------------------------------------------------------------------------


def _fit_exp32_coeffs():
    """Cubic least-squares fit of exp(t)-1 on t = x/256, x in +-7.5 sigma of
    the N(0, 64) score distribution, relative-error weighted. a0 pinned at 1
    so ACT-computed exp and DVE-computed exp agree in absolute scale."""
    t = np.linspace(-90.0, 62.0, 6001) / 256.0
    A = np.stack([t, t * t, t**3], axis=1)
    w = np.exp(-t)
    coef, *_ = np.linalg.lstsq(A * w[:, None], (np.exp(t) - 1.0) * w, rcond=None)
    b1, b2, b3 = (float(c) for c in coef)
    g = 1.0 + b1 * t + b2 * t * t + b3 * t**3
    relerr = float(np.abs(g / np.exp(t) - 1.0).max())
    assert 32.0 * relerr < 5e-3, f"poly too loose: {relerr}"
    return b1 / 256.0, b2 / 256.0**2, b3 / 256.0**3


_C1, _C2, _C3 = _fit_exp32_coeffs()


def _register_dve_ops():
    import concourse.dve_ops as DOPS
    from concourse.dve_spec import (
        C0, C1, C2, One, Spec, Src0, Src1, Zero, _has_src1, lower, maxx,
    )
    from concourse.dve_uop import DveOpSpec

    def ref_poly(in0, in1, c0, c1, c2):
        x = in0.astype(np.float32)
        return np.maximum(
            (x * c2 + c1) * (x * x) + (x * c0 + 1.0), 0.0
        ).astype(np.float32)

    def ref_pow32(in0, in1, c0, c1, c2):
        g = in0.astype(np.float32) ** 32
        if in1 is not None:
            m = np.asarray(in1, np.float32)
            g = g.reshape(m.shape) * m
        return g.astype(np.float32)

    body1 = maxx((Src0 * C2 + C1) * (Src0 * Src0) + (Src0 * C0 + One), Zero)
    g2 = Src0 * Src0
    g4 = g2 * g2
    g8 = g4 * g4
    g16 = g8 * g8
    body2 = (g16 * g16) * Src1

    out = []
    for name, spec in (
        ("ANT_EXP32_POLY", Spec(body=body1, reference=ref_poly)),
        ("ANT_POW32_MASK", Spec(body=body2, reference=ref_pow32)),
    ):
        if name in DOPS._SUB_OPCODE_FOR_NAME:
            out.append(next(op for op in DOPS.OPS if op.name == name))
            continue
        row = max(DOPS._SUB_OPCODE_FOR_NAME.values()) + 1
        assert row < 0x20
        shas = {}
        for ver in ("v3", "v4"):
            try:
                shas[ver] = DveOpSpec(
                    name=name, opcode=row, uops=lower(spec, ver=ver),
                    rd1_en=_has_src1(spec),
                ).sha(ver)
            except Exception:
                pass
        op = DOPS.DveOp(name, spec, subdim=False, uops_sha=shas)
        DOPS.OPS.append(op)
        DOPS.CUSTOM_DVE_SPECS[name] = spec
        DOPS._SUB_OPCODE_FOR_NAME[name] = row
        out.append(op)
    return out


EXP32_POLY, POW32_MASK = _register_dve_ops()

# ---------------------------------------------------------------------------
# Static schedule
# ---------------------------------------------------------------------------

TILE_W = {"A1": 1536, "A2": 1024, "Dt": 1024}
DVE_PANELS = {0: 3, 1: 2, 2: 3, 3: 2}  # extra 8-piece panel chunks on DVE
LOOK = 2  # S-refill lookahead (chunks)


def _build_chunks():
    """Per head: (tile, eng, pieces, diag); pieces = [(kb, c0, c1)] covering
    S^T cols [128c0, 128c1) from k-block kb (row-major strips — one wide S
    matmul per 512-col bank span, minimizing PE instruction count). diag
    chunks carry the 8 diagonal 128-col pieces (kb==c) and run on DVE with
    the causal mask fused; row chunks are fully causal. DVE additionally
    takes DVE_PANELS[h] row chunks, spread uniformly, for exp-load
    balance."""
    heads = []
    for h in range(HPC):
        # row strips (kb, c0, c1): off-diagonal cols of k-block kb
        segs = []  # flat stream of (kb, c0, c1) with c1-c0 <= 8
        for kb in range(15):
            c = kb + 1
            while c < 16:
                take = min(8, 16 - c)
                segs.append((kb, c, c + take))
                c += take
        if h == 0:
            # startup: keep the first two chunks inside the leading DMA
            # pieces (qt[:1152], kt[:256]) by deferring row 0/1's far-column
            # tails a few chunks
            early_tails = [g for g in segs[:4] if g[1] >= 9]
            for g in early_tails:
                segs.remove(g)
                segs.insert(8, g)
        # interleave: diagA ~10% in, diagB ~40%, DVE row chunks offset so
        # DVE work never clusters at head boundaries
        total = sum(c1 - c0 for _, c0, c1 in segs)
        ndve = DVE_PANELS[h]
        chunks = []
        atile = 0
        emitted = 0
        dve_frac = {
            4: [0.18, 0.42, 0.62, 0.82],
            3: [0.30, 0.55, 0.80],
            2: [0.25, 0.70],
            1: [0.55],
            0: [],
        }[ndve]
        if h == HPC - 1:
            dve_frac = [f * 0.82 for f in dve_frac]
        dve_pos = [total * f for f in dve_frac]
        diags = [
            (total * 0.08, [(c, c, c + 1) for c in range(8)]),
            (total * (0.30 if h == HPC - 1 else 0.33),
             [(c, c, c + 1) for c in range(8, 16)]),
        ]
        i = 0
        while i < len(segs):
            if diags and emitted >= diags[0][0]:
                chunks.append(("Dt", "dve", diags.pop(0)[1], True))
                continue
            if dve_pos and emitted >= dve_pos[0]:
                t, eng = "Dt", "dve"
                dve_pos.pop(0)
            else:
                t, eng = ("A1", "A2")[atile % 2], "act"
                atile += 1
            cap = TILE_W[t] // 128
            pieces = []
            used = 0
            while i < len(segs) and used < cap:
                kb, c0, c1 = segs[i]
                take = min(cap - used, c1 - c0)
                pieces.append((kb, c0, c0 + take))
                used += take
                if take < c1 - c0:
                    segs[i] = (kb, c0 + take, c1)
                else:
                    i += 1
            emitted += used
            chunks.append((t, eng, pieces, False))
        for _, d in diags:
            chunks.append(("Dt", "dve", d, True))
        heads.append(chunks)
    return heads


HEAD_CHUNKS = _build_chunks()

_cached = {}


def _build(reps=1):
    key = ("nc", reps)
    if key in _cached:
        return _cached[key]
    nc = bacc.Bacc("TRN2", target_bir_lowering=False, debug=False)
    qt = nc.dram_tensor("qt", (HPC, D, T), BF16, kind="ExternalInput").ap()
    kt = nc.dram_tensor("kt", (HPC, D, T), BF16, kind="ExternalInput").ap()
    # v[h, p, kb, :] = [V[h, 128*kb + p, :] | 1.0]
    v = nc.dram_tensor("v", (HPC, 128, NKB, D + 1), BF16, kind="ExternalInput").ap()
    mask = nc.dram_tensor("mask", (128, 128), BF16, kind="ExternalInput").ap()
    # out [h, p, c*64 + d] = O[h, 128*c + p, d]
    o = nc.dram_tensor("o", (HPC, 128, NKB * D), F32, kind="ExternalOutput").ap()

    with tile.TileContext(nc) as tc:
        with (
            tc.tile_pool(name="constp", bufs=1) as constp,
            tc.tile_pool(name="qkp", bufs=3) as qkp,
            tc.tile_pool(name="pta", bufs=18) as pta,
            tc.tile_pool(name="ptd", bufs=8) as ptd,
            tc.tile_pool(name="gbp", bufs=2) as gbp,
            tc.tile_pool(name="osbp", bufs=2) as osbp,
            tc.tile_pool(name="recp", bufs=6) as recp,
            tc.tile_pool(name="spp", bufs=1, space="PSUM") as spp,
        ):
            mask_sb = constp.tile([128, 128], BF16)
            ones_sb = constp.tile([128, 1], BF16)

            def body():
                _emit_body(
                    nc, tc, qt, kt, v, o, mask_sb, ones_sb, qkp, pta, ptd,
                    gbp, osbp, recp, spp, mask,
                )

            if reps == 1:
                body()
            else:
                with tc.For_i(0, reps, 1):
                    body()

    nc.compile()
    _cached[key] = nc
    return nc


def _emit_body(
    nc, tc, qt, kt, v, o, mask_sb, ones_sb, qkp, pta, ptd, gbp, osbp, recp,
    spp, mask
):
    EXP = mybir.ActivationFunctionType.Exp

    # PSUM (bank-granular tiles, 2 banks each = 8 total): oT holds q-blocks
    # 0-14 (65 cols each); q-block 15 lives in sA2's tail (cols 896:961,
    # within its second bank); sA2's S region is 896 cols.
    oT = spp.tile([128, 512], F32, tag="oT", name="oT")
    sA1 = spp.tile([128, 1536], F32, tag="sA1", name="sA1")
    sDt = spp.tile([128, 1024], F32, tag="sDt", name="sDt")
    sA2 = spp.tile([128, 1024], F32, tag="sA2", name="sA2")
    stile = {"A1": sA1, "A2": sA2, "Dt": sDt}

    sb = {}

    def load(h, first=False):
        qt_sb = qkp.tile([D, T], BF16, tag="qt", name=f"qt_sb{h}")
        kt_sb = qkp.tile([D, T], BF16, tag="kt", name=f"kt_sb{h}")
        v_sb = qkp.tile([128, NKB, D + 1], BF16, tag="v", name=f"v_sb{h}")
        if first:
            # NOTHING on the scalar ring: its HWDGE generations occupy the
            # ACT sequencer (~1.2us each) and head-of-line block the first
            # exp. qt+mask ride sync (SP), kt rides the vector ring (DVE is
            # idle until the first diag chunk), the rest goes SWDGE.
            nc.sync.dma_start(kt_sb[:, :128], kt[h, :, :128])
            nc.sync.dma_start(qt_sb[:, :1152], qt[h, :, :1152])
            nc.sync.dma_start(kt_sb[:, 128:1024], kt[h, :, 128:1024])
            nc.sync.dma_start(qt_sb[:, 1152:], qt[h, :, 1152:])
            nc.gpsimd.dma_start(v_sb[:, : NKB // 2], v[h, :, : NKB // 2])
            nc.gpsimd.dma_start(kt_sb[:, 1024:], kt[h, :, 1024:])
            nc.gpsimd.dma_start(v_sb[:, NKB // 2 :], v[h, :, NKB // 2 :])
        else:
            g = nc.gpsimd
            g.dma_start(kt_sb[:, :1024], kt[h, :, :1024])
            g.dma_start(qt_sb[:, :1024], qt[h, :, :1024])
            g.dma_start(v_sb[:, : NKB // 2], v[h, :, : NKB // 2])
            g.dma_start(kt_sb[:, 1024:], kt[h, :, 1024:])
            g.dma_start(qt_sb[:, 1024:], qt[h, :, 1024:])
            g.dma_start(v_sb[:, NKB // 2 :], v[h, :, NKB // 2 :])
        sb[h] = (qt_sb, kt_sb, v_sb)

    chunks = []
    for h in range(HPC):
        for t, eng, pieces, diag in HEAD_CHUNKS[h]:
            chunks.append((h, t, eng, pieces, diag))
    n = len(chunks)

    ptts = {}

    def emit_S(i):
        h, t, eng, pieces, diag = chunks[i]
        qt_sb, kt_sb, _ = sb[h]
        st = stile[t]
        off = 0
        for kb, c0, c1 in pieces:
            w = 128 * (c1 - c0)
            p = 0
            while p < w:
                # split at tile-local PSUM bank boundaries
                pl = min(w, ((off + p) // 512 + 1) * 512 - off)
                nc.tensor.matmul(
                    st[:, off + p : off + pl],
                    lhsT=kt_sb[:, 128 * kb : 128 * kb + 128],
                    rhs=qt_sb[:, 128 * c0 + p : 128 * c0 + pl],
                    start=True,
                    stop=True,
                )
                p = pl
            off += w

    def emit_exp(i):
        h, t, eng, pieces, diag = chunks[i]
        st = stile[t]
        w = 128 * sum(c1 - c0 for _, c0, c1 in pieces)
        if eng == "act":
            ptt = pta.tile([128, 1536], BF16, tag="pt", name=f"ptt{i}")
            nc.scalar.activation(ptt[:, :w], st[:, :w], EXP, scale=0.125)
        else:
            ptt = ptd.tile([128, 1024], BF16, tag="pt", name=f"ptt{i}")
            gb = gbp.tile([128, 1024], F32, tag="g", name=f"g{i}")
            nc.vector._custom_dve(
                EXP32_POLY, out=gb[:, :w], in0=st[:, :w],
                s0=_C1, s1=_C2, imm2=_C3,
            )
            if diag:
                in1 = mask_sb[:].unsqueeze(1).to_broadcast((128, len(pieces), 128))
                nc.vector._custom_dve(
                    POW32_MASK,
                    out=ptt[:, :w].rearrange("p (a b) -> p a b", b=128),
                    in0=gb[:, :w].rearrange("p (a b) -> p a b", b=128),
                    in1=in1,
                )
            else:
                nc.vector._custom_dve(
                    POW32_MASK, out=ptt[:, :w], in0=gb[:, :w],
                    in1=ones_sb[:].to_broadcast((128, w)),
                )
        # record piece locations for the per-block O bursts
        off = 0
        for kb, c0, c1 in pieces:
            for c in range(c0, c1):
                piece_loc[(h, kb, c)] = (ptt, off)
                off += 128

    piece_loc = {}

    def emit_burst(h, c):
        # PSUM accumulation state is per-bank and survives only while no
        # other start=True hits the bank, so each q-block's O accumulation
        # is emitted as ONE uninterrupted run of matmuls once every
        # contributing P^T piece is available in SBUF.
        _, _, v_sb = sb[h]
        nkb = c + 1  # rows 0..c-1 plus the diagonal piece (kb == c)
        base = 65 * (c % 7)  # rotating 65-col slot in the single O bank
        for idx in range(nkb):
            kb = idx if idx < c else c
            ptt, off = piece_loc.pop((h, kb, c))
            nc.tensor.matmul(
                oT[:, base : base + 65],
                lhsT=ptt[:, off : off + 128],
                rhs=v_sb[:, kb, :],
                start=idx == 0,
                stop=idx == nkb - 1,
                skip_group_check=True,
            )

    osbs = {}

    def emit_norm(h, g):
        # normalize group g of head h: g0 = q-blocks 0-6, g1 = 7-13,
        # g2 = 14-15. Each group's blocks map to contiguous slots in the
        # single O bank (block c -> slot c % 7), so one reciprocal + one
        # fused broadcast multiply + one DMA per group.
        if h not in osbs:
            osbs[h] = osbp.tile([128, NKB * D], F32, tag="osb", name=f"osb{h}")
        o_sb = osbs[h]
        c0, nr = ((0, 7), (7, 7), (14, 2))[g]
        rec = recp.tile([128, 7], F32, tag="rec", name=f"rec{h}_{g}")
        ot65 = oT[:, : 7 * 65].rearrange("p (c e) -> p c e", e=65)
        nc.vector.reciprocal(rec[:, :nr], ot65[:, :nr, 64])
        nc.vector.tensor_mul(
            o_sb[:, 64 * c0 : 64 * (c0 + nr)].rearrange("p (c d) -> p c d", d=64),
            ot65[:, :nr, :64],
            rec[:, :nr].unsqueeze(2).to_broadcast((128, nr, 64)),
        )
        nc.sync.dma_start(
            o[h, :, 64 * c0 : 64 * (c0 + nr)], o_sb[:, 64 * c0 : 64 * (c0 + nr)]
        )

    # PE p-state warm-up: dummy matmuls on a memset tile ramp the tensor
    # engine clock during the input-DMA wait so real S matmuls start fast
    wmm = recp.tile([128, 128], BF16, tag="wmm")
    nc.vector.memset(wmm[:], 0.0)
    for _ in range(6):
        nc.tensor.matmul(
            sA1[:, :128], lhsT=wmm[:], rhs=wmm[:], start=True, stop=True
        )
    load(0, first=True)
    warm = recp.tile([128, 1], F32, tag="warm")
    nc.vector.memset(warm[:], 0.0)
    nc.vector.memset(ones_sb[:], 1.0)
    nc.scalar.activation(warm[:], warm[:], EXP, scale=0.0)
    nc.sync.dma_start(mask_sb[:], mask[:])
    load(1)

    # Per-tile-class S lookahead: ACT tiles deep (LOOK), the DVE tile
    # shallow — a Dt refill in the in-order PE queue waits on DVE's op1
    # (WAR) and would head-block later ACT-tile refills behind it.
    # tile-driven just-in-time S refills: the next chunk on tile T is
    # emitted right after the exp of the current chunk on T (its exact WAR
    # gate), so not-yet-ready matmuls never clog PE's 4-deep wait queue
    next_on_tile = {}
    last_on_tile = {}
    for j, ch in enumerate(chunks):
        if ch[1] in last_on_tile:
            next_on_tile[last_on_tile[ch[1]]] = j
        last_on_tile[ch[1]] = j
    s_done = set()

    def emit_S_once(j):
        if j is not None and j < n and j not in s_done:
            emit_S(j)
            s_done.add(j)

    for tname in ("A1", "A2", "Dt"):
        emit_S_once(next(j for j, ch in enumerate(chunks) if ch[1] == tname))
    remaining = {(h, c): c + 1 for h in range(HPC) for c in range(16)}
    burst_at = {}  # step -> [(h, c)]
    norm_at = {}  # step -> [(h, g)]
    burst_retry = []
    done_norms = set()
    trig = set()
    for i in range(n):
        h, t, eng, pieces, diag = chunks[i]
        emit_exp(i)
        for kb, c0, c1 in pieces:
            for c in range(c0, c1):
                remaining[(h, c)] -= 1
                if remaining[(h, c)] == 0:
                    burst_at.setdefault(i + 1, []).append((h, c))
        # bursts due now are gated on exps already in flight — emit them
        # ahead of this step's S refill (which waits on exp(i)) so PE has
        # ready work during the exp window
        for hg in norm_at.pop(i, []):
            emit_norm(*hg)
            done_norms.add(hg)
        retry = []
        for hc in burst_at.pop(i, []) + burst_retry:
            # the block's O slot (c % 7) is reused across the head; its
            # previous occupant's normalize must be emitted first or this
            # burst's start=True write lands before that read
            hh, c = hc
            if c >= 7:
                prev = (hh, c - 7)
            elif hh > 0:
                prev = (hh - 1, c + 14 if c + 14 <= 15 else c + 7)
            else:
                prev = None
            if prev is not None:
                pg = 0 if prev[1] <= 6 else (1 if prev[1] <= 13 else 2)
                if (prev[0], pg) not in done_norms:
                    retry.append(hc)
                    continue
            emit_burst(*hc)
            if c in (6, 13, 15):
                norm_at.setdefault(i + 1, []).append(
                    (hh, {6: 0, 13: 1, 15: 2}[c])
                )
        burst_retry = retry
        emit_S_once(next_on_tile.get(i))
        if diag and pieces[0][0] == 8 and h + 2 < HPC and ("ld", h + 2) not in trig:
            trig.add(("ld", h + 2))
            load(h + 2)
    pending = burst_retry + [hc for i in sorted(burst_at) for hc in burst_at[i]]
    pending_norms = [hg for i in sorted(norm_at) for hg in norm_at[i]]
    while pending or pending_norms:
        for hg in pending_norms:
            emit_norm(*hg)
            done_norms.add(hg)
        pending_norms = []
        rest = []
        for hh, c in pending:
            if c >= 7:
                prev = (hh, c - 7)
            elif hh > 0:
                prev = (hh - 1, c + 14 if c + 14 <= 15 else c + 7)
            else:
                prev = None
            if prev is not None:
                pg = 0 if prev[1] <= 6 else (1 if prev[1] <= 13 else 2)
                if (prev[0], pg) not in done_norms:
                    rest.append((hh, c))
                    continue
            emit_burst(hh, c)
            if c in (6, 13, 15):
                pending_norms.append((hh, {6: 0, 13: 1, 15: 2}[c]))
        if rest and len(rest) == len(pending) and not pending_norms:
            raise RuntimeError(f"burst deadlock: {rest}")
        pending = rest


def _prep_in_maps(Q, K, V):
    Q = np.asarray(Q, dtype=np.float32).reshape(B * NH, T, D)
    K = np.asarray(K, dtype=np.float32).reshape(B * NH, T, D)
    V = np.asarray(V, dtype=np.float32).reshape(B * NH, T, D)

    mask = np.where(
        np.arange(128)[:, None] <= np.arange(128)[None, :], 1.0, 0.0
    ).astype(ml_dtypes.bfloat16)

    in_maps = []
    for cc in range(NCORES):
        hs = slice(HPC * cc, HPC * (cc + 1))
        qtc = np.ascontiguousarray(Q[hs].transpose(0, 2, 1)).astype(
            ml_dtypes.bfloat16
        )
        ktc = np.ascontiguousarray(K[hs].transpose(0, 2, 1)).astype(
            ml_dtypes.bfloat16
        )
        va = np.concatenate(
            [V[hs], np.ones((HPC, T, 1), dtype=np.float32)], axis=-1
        )
        vc = np.ascontiguousarray(
            va.reshape(HPC, NKB, 128, D + 1).transpose(0, 2, 1, 3)
        ).astype(ml_dtypes.bfloat16)
        in_maps.append({"qt": qtc, "kt": ktc, "v": vc, "mask": mask})
    return in_maps


def _gather(results):
    out = np.empty((B * NH, T, D), dtype=np.float32)
    for cc in range(NCORES):
        oc = results[cc]["o"]  # [HPC, 128, NKB*D]
        for s in range(HPC):
            out[HPC * cc + s] = (
                oc[s].reshape(128, NKB, D).transpose(1, 0, 2).reshape(T, D)
            )
    return out.reshape(B, NH, T, D)


def _run(in_maps, **kwargs):
    nc = _build()
    return run_bass_kernel_spmd(nc, in_maps, core_ids=list(range(NCORES)), **kwargs)


def kernel(Q, K, V):
    in_maps = _prep_in_maps(Q, K, V)
    res = _run(in_maps)
    return _gather(res.results)
